# revision 28
# baseline (speedup 1.0000x reference)
"""DANetHead (dual attention) Trainium2 kernel.

Full inputs in, full outputs out. Internally sharded over 8 NeuronCores:
core c -> batch b=c//4, row-slice s=c%4 (16 rows of the 64x64 image).
Two SPMD launches with host-side reshuffle between them:
  launch1: fused 3x3 conv (2048->1024: PA&CA branch convs together, f32r) +
           BN+ReLU, q/k 1x1 (f32r), v^T (bf16, transposed form), partial
           channel Gram matrix (f32r mm, fp32 accum; summed on host).
  launch2: PAM attention (transpose-free two-pass softmax), CAM channel
           attention, output convs (bf16), classifiers, fusion.

PAM softmax without transposing the attention matrix:
  pass1 (row-major e[i,j], bf16): per-query max M_i only. bf16 logit noise
        (+-8 of ~1.8e3) is harmless here: M only shifts the exps.
  pass2 (column-major e^T[j,i], fp32): contraction augmented to 65 rows
        (k' = [k; 1], q' = [q; -M]) so the PE emits e^T - M directly; ACT
        exps it straight into the [key, query] layout that the PA matmul
        consumes with v^T as lhsT -- no PE transposes, no DVE copies.
        Numerators need full fp32: f32r operand truncation on q/k (~+-30)
        gives +-3 logit noise, which scrambles the near-one-hot softmax.
  denominators: ones-column matmul over the same exp'd pT accumulated
        alongside the PA matmuls -- exactly consistent with numerators.
  normalization (gamma * mask / S) folds into the epilogue via a ones-lhsT
        broadcast matmul.

Precision elsewhere: f32r for the big convs / q/k projection / Gram
(validated: ca_out err 4.5e-3), bf16 after the softmaxes.
"""

import sys

sys.path.insert(0, "/opt/trn_rl_repo")

import numpy as np
import ml_dtypes

import concourse.bass as bass
import concourse.mybir as mybir
import concourse.tile as tile
from concourse import bacc
from concourse.bass_utils import run_bass_kernel_spmd
from concourse.masks import make_identity

BF16 = mybir.dt.bfloat16
F32 = mybir.dt.float32
F32R = mybir.dt.float32r
AF = mybir.ActivationFunctionType
ALU = mybir.AluOpType

B, CIN, H, W, NCLS = 2, 2048, 64, 64, 19
CI = 512          # inter channels
C8 = 64           # q/k channels
N = H * W         # 4096 pixels per image
NCORE = 8
S = 4             # row slices per batch
RS = H // S       # 16 rows per slice
HR = RS + 2       # 18 rows incl. halo
NPIX = RS * W     # 1024 pixels per slice
NPIXH = HR * W    # 1152 pixels incl. halo (the query set)
NT3 = NPIXH // 384  # 3 thirds of 384 queries
EPS = 1e-5

bf16 = ml_dtypes.bfloat16


# --------------------------------------------------------------------------
# launch 1: conv(2048 -> 1024, 3x3, f32r) + BN + ReLU ; qk(f32r) ; vT ; cen
# --------------------------------------------------------------------------

def build_launch1():
    nc = bacc.Bacc(None, target_bir_lowering=False)

    XP = nc.dram_tensor("XP", [128, 16, HR, W + 2], F32R, kind="ExternalInput")
    W1T = nc.dram_tensor("W1T", [8, 128, 16, 9, 128], F32R, kind="ExternalInput")
    FGSC = nc.dram_tensor("FGSC", [128, 8], F32, kind="ExternalInput")
    FGSH = nc.dram_tensor("FGSH", [128, 8], F32, kind="ExternalInput")
    QKWT = nc.dram_tensor("QKWT", [4, 128, 128], F32R, kind="ExternalInput")
    QKB = nc.dram_tensor("QKB", [128, 1], F32, kind="ExternalInput")
    VWT = nc.dram_tensor("VWT", [4, 128, 512], BF16, kind="ExternalInput")
    IDR = nc.dram_tensor("IDR", [128, 128], F32R, kind="ExternalInput")

    FG = nc.dram_tensor("FG", [8, 128, RS, W], F32R, kind="ExternalOutput")
    QK = nc.dram_tensor("QK", [128, NPIX], F32, kind="ExternalOutput")
    VT = nc.dram_tensor("VT", [8, 128, 512], BF16, kind="ExternalOutput")
    CENP = nc.dram_tensor("CENP", [4, 128, 512], F32, kind="ExternalOutput")

    with tile.TileContext(nc) as tc:
        with (
            tc.tile_pool(name="singles", bufs=1) as singles,
            tc.tile_pool(name="wpool", bufs=2) as wpool,
            tc.tile_pool(name="opool", bufs=2) as opool,
            tc.tile_pool(name="pspool", bufs=2, space="PSUM") as pspool,
        ):
            x_all = singles.tile([128, 16, HR, W + 2], F32R)

            # first conv chunk (x + weights) lands before anything else
            wv00 = wpool.tile([128, 4, 9, 128], F32R, tag="w")
            for t in range(4):
                nc.sync.dma_start(x_all[:, t], XP[:, t])
                nc.gpsimd.dma_start(wv00[:, t], W1T[4][:, t])

            fgsc = singles.tile([128, 8], F32)
            nc.sync.dma_start(fgsc[:], FGSC[:])
            fgsh = singles.tile([128, 8], F32)
            nc.sync.dma_start(fgsh[:], FGSH[:])
            qkwt = singles.tile([128, 4, 128], F32R)
            nc.sync.dma_start(qkwt[:], QKWT.ap().rearrange("t p c -> p t c"))
            qkb = singles.tile([128, 1], F32)
            nc.sync.dma_start(qkb[:], QKB[:])
            vwt = singles.tile([128, 4, 512], BF16)
            nc.sync.dma_start(vwt[:], VWT.ap().rearrange("t p c -> p t c"))
            identr = singles.tile([128, 128], F32R)
            nc.sync.dma_start(identr[:], IDR[:])

            # conv outputs: f32r resident (qk/cen matmuls) + bf16 copy (vt)
            fgout32 = singles.tile([128, 8, RS, W], F32R)
            fg_bf = singles.tile([128, 4, RS, W], BF16)
            gt32 = singles.tile([128, 8, 512], F32R)  # g^T per 128-px tile

            fgv32 = fgout32.rearrange("p t r c -> p t (r c)")
            fgv = fg_bf.rearrange("p t r c -> p t (r c)")

            def conv_cot(cot, first=False):
                acc2 = pspool.tile([128, 2, 8, W], F32, tag="conv")
                for ch in range(4):
                    if first and ch == 0:
                        wv = wv00
                    else:
                        wv = wpool.tile([128, 4, 9, 128], F32R, tag="w")
                        if first:
                            nc.sync.dma_start(
                                x_all[:, ch * 4:(ch + 1) * 4],
                                XP[:, ch * 4:(ch + 1) * 4],
                            )
                        nc.gpsimd.dma_start(wv[:], W1T[cot][:, ch * 4:(ch + 1) * 4])
                    for rb in range(2):
                        for cit4 in range(4):
                            for dd in range(9):
                                dy, dx = dd // 3, dd % 3
                                r0 = rb * 8 + dy
                                nc.tensor.matmul(
                                    acc2[:, rb],
                                    wv[:, cit4, dd, :],
                                    x_all[:, ch * 4 + cit4, r0:r0 + 8, dx:dx + W],
                                    start=(ch == 0 and cit4 == 0 and dd == 0),
                                    stop=(ch == 3 and cit4 == 3 and dd == 8),
                                )
                for rb in range(2):
                    nc.scalar.activation(
                        out=fgout32[:, cot, rb * 8:(rb + 1) * 8, :],
                        in_=acc2[:, rb],
                        func=AF.Relu,
                        bias=fgsh[:, cot:cot + 1],
                        scale=fgsc[:, cot:cot + 1],
                    )
                    if cot < 4:
                        nc.vector.tensor_copy(
                            fg_bf[:, cot, rb * 8:(rb + 1) * 8, :],
                            fgout32[:, cot, rb * 8:(rb + 1) * 8, :],
                        )
                    else:
                        # g^T transposes as soon as each 8-row block lands
                        ct = cot - 4
                        for nt in range(rb * 4, rb * 4 + 4):
                            tp = pspool.tile([128, 128], F32R, tag="small")
                            nc.tensor.transpose(
                                tp[:], fgv32[:, cot, nt * 128:(nt + 1) * 128],
                                identr[:],
                            )
                            nc.vector.tensor_copy(
                                gt32[:, nt, ct * 128:(ct + 1) * 128], tp[:]
                            )
                    nc.sync.dma_start(
                        FG[cot, :, rb * 8:(rb + 1) * 8, :],
                        fgout32[:, cot, rb * 8:(rb + 1) * 8, :],
                    )

            # ---- g branch convs first (transposes inlined per row-block);
            #      Gram halves then hide under the first f convs ----
            conv_cot(4, first=True)
            for ct in range(1, 4):
                conv_cot(4 + ct)

            cen_sb = opool.tile([128, 4, 512], F32, tag="cen_sb", bufs=1)

            def gram_half(half):
                cen_ps = pspool.tile([128, 2, 512], F32, tag="qkcen", bufs=1)
                for nt in range(8):
                    for ct2 in range(2):
                        ct = half * 2 + ct2
                        nc.tensor.matmul(
                            cen_ps[:, ct2, :],
                            gt32[:, nt, ct * 128:(ct + 1) * 128],
                            gt32[:, nt, :],
                            start=(nt == 0),
                            stop=(nt == 7),
                        )
                nc.vector.tensor_copy(
                    cen_sb[:, half * 2:(half + 1) * 2], cen_ps[:]
                )

            # ---- f branch convs, Gram interleaved ----
            conv_cot(0)
            gram_half(0)
            conv_cot(1)
            gram_half(1)
            nc.sync.dma_start(CENP.ap().rearrange("t p c -> p t c"), cen_sb[:])
            conv_cot(2)
            conv_cot(3)

            # ---- q/k : packed f32r matmul (q rows 0:64, k rows 64:128) ----
            qk_ps = pspool.tile([128, 2, 512], F32, tag="qkcen", bufs=1)
            for ck in range(2):
                for cit in range(4):
                    nc.tensor.matmul(
                        qk_ps[:, ck, :],
                        qkwt[:, cit, :],
                        fgv32[:, cit, ck * 512:(ck + 1) * 512],
                        start=(cit == 0),
                        stop=(cit == 3),
                    )
            qk_sb = opool.tile([128, NPIX], F32, tag="qk_sb")
            nc.vector.tensor_scalar(
                out=qk_sb[:], in0=qk_ps.rearrange("p a b -> p (a b)"),
                scalar1=qkb[:], scalar2=None, op0=ALU.add,
            )
            nc.sync.dma_start(QK[:], qk_sb[:])

            # ---- vT[n, c] (no bias: folded in launch2) ----
            for nt in range(8):
                vps = pspool.tile([128, 512], F32, tag="small")
                for cit in range(4):
                    nc.tensor.matmul(
                        vps[:],
                        fgv[:, cit, nt * 128:(nt + 1) * 128],
                        vwt[:, cit, :],
                        start=(cit == 0),
                        stop=(cit == 3),
                    )
                vt_sb = opool.tile([128, 512], BF16, tag="vt_sb")
                nc.vector.tensor_copy(vt_sb[:], vps[:])
                nc.sync.dma_start(VT[nt], vt_sb[:])

    nc.compile()
    return nc


# --------------------------------------------------------------------------
# launch 2: PAM (transpose-free) + CAM + output convs + classifiers + fusion
# --------------------------------------------------------------------------

def build_launch2():
    nc = bacc.Bacc(None, target_bir_lowering=False)

    KAUG = nc.dram_tensor("KAUG", [65, N], F32, kind="ExternalInput")
    QAUG = nc.dram_tensor("QAUG", [65, NPIXH], F32, kind="ExternalInput")
    KB16 = nc.dram_tensor("KB16", [64, N], BF16, kind="ExternalInput")
    QB16 = nc.dram_tensor("QB16", [64, NPIXH], BF16, kind="ExternalInput")
    VT2 = nc.dram_tensor("VT2", [32, 128, 512], BF16, kind="ExternalInput")
    CEN = nc.dram_tensor("CEN", [4, 128, 512], F32, kind="ExternalInput")
    FH = nc.dram_tensor("FH", [4, 128, HR, W], BF16, kind="ExternalInput")
    GH = nc.dram_tensor("GH", [4, 128, HR, W], BF16, kind="ExternalInput")
    W2T = nc.dram_tensor("W2T", [2, 4, 128, 4, 9, 128], BF16, kind="ExternalInput")
    OSC = nc.dram_tensor("OSC", [128, 8], F32, kind="ExternalInput")
    OSH = nc.dram_tensor("OSH", [128, 8], F32, kind="ExternalInput")
    CLSW = nc.dram_tensor("CLSW", [3, 4, 128, NCLS], BF16, kind="ExternalInput")
    CLSB = nc.dram_tensor("CLSB", [NCLS, 3], F32, kind="ExternalInput")
    VB = nc.dram_tensor("VB", [128, 4], F32, kind="ExternalInput")
    GAM = nc.dram_tensor("GAM", [1, 2], F32, kind="ExternalInput")
    MSKB = nc.dram_tensor("MSKB", [128, NPIXH], BF16, kind="ExternalInput")

    OUT = nc.dram_tensor("OUT", [3, NCLS, RS, W], F32, kind="ExternalOutput")

    with tile.TileContext(nc) as tc:
        with (
            tc.tile_pool(name="singles", bufs=1) as singles,
            tc.tile_pool(name="w2p", bufs=2) as w2p,
            tc.tile_pool(name="work", bufs=2) as work,
            tc.tile_pool(name="cols", bufs=4) as cols,
            tc.tile_pool(name="pspool", bufs=2, space="PSUM") as pspool,
        ):
            # ---- input DMAs, roughly in order of first use ----
            qb16 = singles.tile([64, NPIXH], BF16)
            nc.sync.dma_start(qb16[:], QB16[:])
            kb16 = singles.tile([64, N], BF16)
            nc.sync.dma_start(kb16[:], KB16[:])
            cen = singles.tile([128, 4, 512], F32)
            nc.sync.dma_start(cen[:], CEN.ap().rearrange("t p c -> p t c"))
            gh = singles.tile([128, 4, HR, W], BF16)
            nc.sync.dma_start(gh[:], GH.ap().rearrange("t p r c -> p t r c"))
            gam_pa = singles.tile([128, 1], F32)
            nc.sync.dma_start(
                gam_pa[:],
                bass.AP(tensor=GAM.ap().tensor, offset=0, ap=[[0, 128], [1, 1]]),
            )
            gam_ca = singles.tile([128, 1], F32)
            nc.sync.dma_start(
                gam_ca[:],
                bass.AP(tensor=GAM.ap().tensor, offset=1, ap=[[0, 128], [1, 1]]),
            )
            vb = singles.tile([128, 4], F32)
            nc.sync.dma_start(vb[:], VB[:])
            osc = singles.tile([128, 8], F32)
            nc.sync.dma_start(osc[:], OSC[:])
            osh = singles.tile([128, 8], F32)
            nc.sync.dma_start(osh[:], OSH[:])
            qaug = singles.tile([65, NPIXH], F32)
            nc.sync.dma_start(qaug[0:64, :], QAUG[0:64, :])
            kaug = singles.tile([65, N], F32)
            mskb = singles.tile([128, NPIXH], BF16)
            fh = singles.tile([128, 4, HR, W], BF16)
            vt = singles.tile([128, 32, 512], BF16)
            clsw = singles.tile([128, 3, 4, NCLS], BF16)
            ones1 = singles.tile([1, 128], F32)
            nc.sync.dma_start(ones1[:], KAUG[64:65, 0:128])
            clsb = singles.tile([NCLS, 3], F32)
            nc.sync.dma_start(clsb[:], CLSB[:])

            identf = singles.tile([128, 128], F32)
            make_identity(nc, identf[:])
            identb = singles.tile([128, 128], BF16)
            make_identity(nc, identb[:])
            onesj = singles.tile([128, 1], BF16)
            nc.vector.memset(onesj[:], 1.0)

            ghv = gh.rearrange("p t r c -> p t (r c)")
            fhv = fh.rearrange("p t r c -> p t (r c)")

            gvb = singles.tile([128, 4], F32)
            nc.vector.tensor_scalar(
                out=gvb[:], in0=vb[:], scalar1=gam_pa[:], scalar2=None,
                op0=ALU.mult,
            )

            negm9 = singles.tile([128, 9], F32)
            feat_bf = singles.tile([128, 2, 4, RS, W], BF16)
            pT = singles.tile([128, 32, 384], BF16)
            pabuf = singles.tile([128, 4, HR, W + 2], BF16)
            cabuf = singles.tile([128, 4, HR, W + 2], BF16)
            nc.vector.memset(cabuf[:], 0.0)

            # ---- PAM pass 1: bf16 row-major energies, per-query max ----
            def pass1_it(it):
                nmx8 = cols.tile([128, 8], F32, tag="nmx8")
                for jc in range(8):
                    eps = pspool.tile([128, 512], F32, tag="e")
                    nc.tensor.matmul(
                        eps[:],
                        qb16[:, it * 128:(it + 1) * 128],
                        kb16[:, jc * 512:(jc + 1) * 512],
                        start=True, stop=True,
                    )
                    nc.vector.tensor_reduce(
                        out=nmx8[:, jc:jc + 1], in_=eps[:], op=ALU.max,
                        axis=mybir.AxisListType.X, negate=True,
                    )
                nc.vector.tensor_reduce(
                    out=negm9[:, it:it + 1], in_=nmx8[:], op=ALU.min,
                    axis=mybir.AxisListType.X,
                )

            def third_negm(k):
                # -M into qaug row 64 (3 single-partition DMAs)
                tpn = pspool.tile([3, 128], F32, tag="e")
                nc.tensor.transpose(tpn[:], negm9[:, 3 * k:3 * k + 3], identf[:])
                rowr = work.tile([3, 128], F32, tag="rowr")
                nc.vector.tensor_copy(rowr[:], tpn[:])
                for a in range(3):
                    nc.sync.dma_start(
                        qaug[64:65, 384 * k + 128 * a:384 * k + 128 * (a + 1)],
                        rowr[a:a + 1, :],
                    )

            def ca_branch():
                E_sb = singles.tile([128, 4, 512], BF16)
                Scol = singles.tile([128, 4], F32)
                for ct in range(4):
                    mn = cols.tile([128, 1], F32, tag="camn")
                    nc.vector.tensor_reduce(
                        out=mn[:], in_=cen[:, ct, :], op=ALU.min,
                        axis=mybir.AxisListType.X,
                    )
                    nc.scalar.activation(
                        out=E_sb[:, ct, :], in_=cen[:, ct, :], func=AF.Exp,
                        bias=mn[:], scale=-1.0, accum_out=Scol[:, ct:ct + 1],
                    )
                grS = singles.tile([128, 4], F32)
                nc.vector.reciprocal(grS[:], Scol[:])
                nc.vector.tensor_scalar(
                    out=grS[:], in0=grS[:], scalar1=gam_ca[:], scalar2=None,
                    op0=ALU.mult,
                )
                ET = singles.tile([128, 4, 512], BF16)
                for ct in range(4):
                    for dt in range(4):
                        tp = pspool.tile([128, 128], BF16, tag="e")
                        nc.tensor.transpose(
                            tp[:], E_sb[:, ct, dt * 128:(dt + 1) * 128],
                            identb[:],
                        )
                        nc.vector.tensor_copy(
                            ET[:, dt, ct * 128:(ct + 1) * 128], tp[:]
                        )
                for ck in range(3):
                    px0 = ck * 384
                    ca_ps = pspool.tile([128, 4, 512], F32, tag="pa", bufs=1)
                    for ct in range(4):
                        for dt in range(4):
                            nc.tensor.matmul(
                                ca_ps[:, ct, :384],
                                ET[:, dt, ct * 128:(ct + 1) * 128],
                                ghv[:, dt, px0:px0 + 384],
                                start=(dt == 0),
                                stop=(dt == 3),
                            )
                    for ct in range(4):
                        tmp = work.tile([128, 384], F32, tag="catmp")
                        nc.vector.tensor_scalar(
                            out=tmp[:], in0=ca_ps[:, ct, :384],
                            scalar1=grS[:, ct:ct + 1], scalar2=None,
                            op0=ALU.mult,
                        )
                        nc.vector.tensor_add(
                            cabuf[:, ct, ck * 6:(ck + 1) * 6, 1:1 + W],
                            tmp.rearrange("p (r c) -> p r c", c=W),
                            ghv[:, ct, px0:px0 + 384]
                            .rearrange("p (r c) -> p r c", c=W),
                        )

            def w2_load(br, cot):
                w2v = w2p.tile([128, 4, 9, 128], BF16, tag="w2")
                nc.gpsimd.dma_start(w2v[:], W2T[br, cot])
                return w2v

            def conv_group(br, buf, cot, rb, w2v):
                acc = pspool.tile([128, 8, W], F32, tag="cv")
                nmm = 0
                for cit in range(4):
                    wq = w2v[:, cit]
                    for dd in range(9):
                        dy, dx = dd // 3, dd % 3
                        r0 = rb * 8 + dy
                        nc.tensor.matmul(
                            acc[:],
                            wq[:, dd, :],
                            buf[:, cit, r0:r0 + 8, dx:dx + W],
                            start=(nmm == 0),
                            stop=(nmm == 35),
                        )
                        nmm += 1
                nc.scalar.activation(
                    out=feat_bf[:, br, cot, rb * 8:(rb + 1) * 8, :],
                    in_=acc[:],
                    func=AF.Relu,
                    bias=osh[:, br * 4 + cot:br * 4 + cot + 1],
                    scale=osc[:, br * 4 + cot:br * 4 + cot + 1],
                )

            featv = feat_bf.rearrange("p b t r c -> p b t (r c)")

            def classifier_ck(which, ck):
                # which 0: fusion (paf + caf through fW), 1: pa, 2: ca
                # ck 0 covers rows 0..7 (rb0 features), ck 1 rows 8..15
                sl = slice(ck * 512, (ck + 1) * 512)
                cls_ps = pspool.tile([NCLS, 512], F32, tag="cv")
                if which == 0:
                    nmm = 0
                    for br in range(2):
                        for cit in range(4):
                            nc.tensor.matmul(
                                cls_ps[:],
                                clsw[:, 0, cit, :],
                                featv[:, br, cit, sl],
                                start=(nmm == 0), stop=(nmm == 7),
                            )
                            nmm += 1
                else:
                    br = which - 1
                    for cit in range(4):
                        nc.tensor.matmul(
                            cls_ps[:],
                            clsw[:, which, cit, :],
                            featv[:, br, cit, sl],
                            start=(cit == 0), stop=(cit == 3),
                        )
                out_sb = work.tile([NCLS, 512], F32, tag="out_sb")
                nc.vector.tensor_scalar(
                    out=out_sb[:], in0=cls_ps[:],
                    scalar1=clsb[:, which:which + 1], scalar2=None,
                    op0=ALU.add,
                )
                nc.sync.dma_start(
                    OUT[which, :, ck * 8:(ck + 1) * 8, :]
                    .rearrange("p r c -> p (r c)"),
                    out_sb[:],
                )

            # ---- pass1 its interleaved with CA branch + cao conv ----
            cao_groups = [(cot, rb) for cot in range(4) for rb in range(2)]
            w2v_live = {}

            def cao_group(g):
                cot, rb = cao_groups[g]
                if rb == 0:
                    w2v_live[cot] = w2_load(1, cot)
                conv_group(1, cabuf, cot, rb, w2v_live[cot])

            pass1_it(0)
            pass1_it(1)
            ca_branch()
            pass1_it(2)
            third_negm(0)
            for it in range(3, 9):
                pass1_it(it)
                if it == 5:
                    third_negm(1)
                elif it == 8:
                    third_negm(2)
                cao_group(it - 3)

            # big late-use inputs: issued behind the cao weight tiles on
            # the gpsimd queue, all needed >=60us into the kernel
            nc.gpsimd.dma_start(kaug[:], KAUG[:])
            nc.gpsimd.dma_start(vt[:], VT2.ap().rearrange("n p c -> p n c"))
            nc.gpsimd.dma_start(fh[:], FH.ap().rearrange("t p r c -> p t r c"))
            nc.gpsimd.dma_start(mskb[:], MSKB[:])
            nc.gpsimd.dma_start(clsw[:], CLSW.ap().rearrange("w t p c -> p w t c"))

            nc.vector.memset(pabuf[:], 0.0)

            # FM = (f + gamma*vb) * mask  (residual term of the PA epilogue)
            FM = singles.tile([128, 4, NPIXH], BF16)
            for ct in range(4):
                nc.vector.tensor_scalar(
                    out=FM[:, ct, :], in0=fhv[:, ct, :],
                    scalar1=gvb[:, ct:ct + 1], scalar2=None, op0=ALU.add,
                )
                nc.vector.tensor_mul(FM[:, ct, :], FM[:, ct, :], mskb[:])

            # ---- PAM pass 2 + PA accumulation, per third ----
            def eT_exp(k, jt):
                eT = pspool.tile([128, 384], F32, tag="e")
                nc.tensor.matmul(
                    eT[:],
                    kaug[:, jt * 128:(jt + 1) * 128],
                    qaug[:, 384 * k:384 * (k + 1)],
                    start=True, stop=True,
                )
                nc.scalar.activation(
                    out=pT[:, jt, :], in_=eT[:], func=AF.Exp,
                    bias=0.0, scale=1.0,
                )

            for k in range(NT3):
                q_sl = slice(384 * k, 384 * (k + 1))
                if k == 0:
                    for jt in range(16):
                        eT_exp(0, jt)
                    cao_group(6)
                    for jt in range(16, 32):
                        eT_exp(0, jt)
                    cao_group(7)
                    classifier_ck(2, 0)     # ca head: only needs cao output
                    classifier_ck(2, 1)
                    w2v_live[0] = w2_load(0, 0)
                    w2v_live[1] = w2_load(0, 1)
                pa_ps = pspool.tile([128, 4, 512], F32, tag="pa", bufs=1)
                dn = pspool.tile([1, 384], F32, tag="cv")
                for jt in range(32):
                    for ct in range(4):
                        nc.tensor.matmul(
                            pa_ps[:, ct, :384],
                            vt[:, jt, ct * 128:(ct + 1) * 128],
                            pT[:, jt, :],
                            start=(jt == 0),
                            stop=(jt == 31),
                        )
                    nc.tensor.matmul(
                        dn[:], onesj[:], pT[:, jt, :],
                        start=(jt == 0), stop=(jt == 31),
                    )
                    if k + 1 < NT3:
                        eT_exp(k + 1, jt)
                # R = gamma * mask / S, broadcast over channel partitions
                R3r = work.tile([1, 384], F32, tag="R3r")
                nc.vector.reciprocal(R3r[:], dn[:])
                nc.vector.tensor_scalar(
                    out=R3r[:], in0=R3r[:], scalar1=gam_pa[0:1, :],
                    scalar2=None, op0=ALU.mult,
                )
                nc.vector.tensor_mul(R3r[:], R3r[:], mskb[0:1, q_sl])
                rb_ps = pspool.tile([128, 384], F32, tag="e")
                nc.tensor.matmul(rb_ps[:], ones1[:], R3r[:],
                                 start=True, stop=True)
                Rbm = work.tile([128, 384], F32, tag="Rbm")
                nc.vector.tensor_copy(Rbm[:], rb_ps[:])
                # epilogue: pabuf = pa * R + FM
                for ct in range(4):
                    tmp2 = work.tile([128, 384], F32, tag="patmp")
                    nc.vector.tensor_mul(tmp2[:], pa_ps[:, ct, :384], Rbm[:])
                    nc.vector.tensor_add(
                        pabuf[:, ct, k * 6:(k + 1) * 6, 1:1 + W],
                        tmp2.rearrange("p (r c) -> p r c", c=W),
                        FM[:, ct, q_sl].rearrange("p (r c) -> p r c", c=W),
                    )
                if k == 1:
                    # pao rb0: needs pabuf rows 0..9 (thirds 0 and 1)
                    conv_group(0, pabuf, 0, 0, w2v_live[0])
                    w2v_live[2] = w2_load(0, 2)
                    conv_group(0, pabuf, 1, 0, w2v_live[1])
                    w2v_live[3] = w2_load(0, 3)
                    conv_group(0, pabuf, 2, 0, w2v_live[2])
                    conv_group(0, pabuf, 3, 0, w2v_live[3])
                    classifier_ck(0, 0)     # rows 0..7 ready
                    classifier_ck(1, 0)
                elif k == 2:
                    w2v_live[0] = w2_load(0, 0)
                    w2v_live[1] = w2_load(0, 1)

            # pao rb1 (pabuf rows 8..17)
            conv_group(0, pabuf, 0, 1, w2v_live[0])
            w2v_live[2] = w2_load(0, 2)
            conv_group(0, pabuf, 1, 1, w2v_live[1])
            w2v_live[3] = w2_load(0, 3)
            conv_group(0, pabuf, 2, 1, w2v_live[2])
            conv_group(0, pabuf, 3, 1, w2v_live[3])

            classifier_ck(0, 1)
            classifier_ck(1, 1)

    nc.compile()
    return nc


# --------------------------------------------------------------------------
# host-side preparation and glue
# --------------------------------------------------------------------------

_CACHE = {}


def _get_kernels():
    if "nc1" not in _CACHE:
        _CACHE["nc1"] = build_launch1()
        _CACHE["nc2"] = build_launch2()
    return _CACHE["nc1"], _CACHE["nc2"]


def _fold_bn(g, b, m, v, conv_b):
    scale = g / np.sqrt(v + EPS)
    shift = (conv_b - m) * scale + b
    return scale.astype(np.float32), shift.astype(np.float32)


def _prep_launch1(x, paW, pab, pa_bn, caW, cab, ca_bn, qW, qb, kW, kb, vW):
    """Build the 8 per-core input maps for launch 1."""
    W1 = np.concatenate([paW, caW], axis=0)            # (1024, 2048, 3, 3)
    w1t = np.ascontiguousarray(
        np.transpose(W1.reshape(8, 128, 16, 128, 3, 3), (0, 3, 2, 4, 5, 1))
    ).reshape(8, 128, 16, 9, 128).astype(np.float32)

    sc_f, sh_f = _fold_bn(*pa_bn, pab)
    sc_g, sh_g = _fold_bn(*ca_bn, cab)
    fgsc = np.concatenate([sc_f, sc_g]).reshape(8, 128).T.copy()   # (128, 8)
    fgsh = np.concatenate([sh_f, sh_g]).reshape(8, 128).T.copy()

    qkW = np.concatenate([qW[:, :, 0, 0], kW[:, :, 0, 0]], axis=0)  # (128, 512)
    qkwt = np.ascontiguousarray(
        qkW.T.reshape(4, 128, 128)
    ).astype(np.float32)                               # [cit, ci, co]
    qkb_ = np.concatenate([qb, kb]).reshape(128, 1).astype(np.float32)
    vwt = np.ascontiguousarray(
        vW[:, :, 0, 0].T.reshape(4, 128, 512)
    ).astype(bf16)
    idr = np.eye(128, dtype=np.float32)

    # padded input slices, pre-transposed to partition-major layout
    xpad = np.zeros((B, CIN, H + 2, W + 2), dtype=np.float32)
    xpad[:, :, 1:H + 1, 1:W + 1] = x.astype(np.float32)

    in_maps = []
    for c in range(NCORE):
        b_, s_ = divmod(c, S)
        rows = slice(s_ * RS, s_ * RS + HR)            # in padded coords
        xp = np.ascontiguousarray(
            xpad[b_, :, rows, :].reshape(16, 128, HR, W + 2)
            .transpose(1, 0, 2, 3)
        )
        in_maps.append({
            "XP": xp, "W1T": w1t, "FGSC": fgsc, "FGSH": fgsh,
            "QKWT": qkwt, "QKB": qkb_, "VWT": vwt, "IDR": idr,
        })
    return in_maps


def _prep_launch2(r1, paoW, paob, pao_bn, caoW, caob, cao_bn,
                  paclsW, paclsb, caclsW, caclsb, fW, fb,
                  vb, pam_gamma, cam_gamma):
    """Reshuffle launch-1 outputs and build launch-2 input maps."""
    f_full = np.zeros((B, 4, 128, H, W), dtype=np.float32)
    g_full = np.zeros((B, 4, 128, H, W), dtype=np.float32)
    q_full = np.zeros((B, 64, H, W), dtype=np.float32)
    k_full = np.zeros((B, 64, H, W), dtype=np.float32)
    vt_full = np.zeros((B, 32, 128, 512), dtype=bf16)
    cen_full = np.zeros((B, 4, 128, 512), dtype=np.float32)
    for c in range(NCORE):
        b_, s_ = divmod(c, S)
        r = r1[c]
        rows = slice(s_ * RS, (s_ + 1) * RS)
        f_full[b_, :, :, rows, :] = r["FG"][0:4]
        g_full[b_, :, :, rows, :] = r["FG"][4:8]
        qk = r["QK"].reshape(128, RS, W)
        q_full[b_, :, rows, :] = qk[0:64]
        k_full[b_, :, rows, :] = qk[64:128]
        vt_full[b_, s_ * 8:(s_ + 1) * 8] = r["VT"]
        cen_full[b_] += r["CENP"]

    w2 = np.stack([paoW, caoW])                        # (2, 512, 512, 3, 3)
    w2t = np.ascontiguousarray(
        np.transpose(w2.reshape(2, 4, 128, 4, 128, 3, 3), (0, 1, 4, 3, 5, 6, 2))
    ).reshape(2, 4, 128, 4, 9, 128).astype(bf16)

    sc_p, sh_p = _fold_bn(*pao_bn, paob)
    sc_c, sh_c = _fold_bn(*cao_bn, caob)
    osc = np.concatenate([sc_p, sc_c]).reshape(8, 128).T.copy()
    osh = np.concatenate([sh_p, sh_c]).reshape(8, 128).T.copy()

    clsw = np.stack([
        fW[:, :, 0, 0], paclsW[:, :, 0, 0], caclsW[:, :, 0, 0]
    ])                                                 # (3, 19, 512)
    clsw_t = np.ascontiguousarray(
        np.transpose(clsw.reshape(3, NCLS, 4, 128), (0, 2, 3, 1))
    ).astype(bf16)                                     # (3, 4, 128, 19)
    clsb = np.stack([fb, paclsb, caclsb], axis=1).astype(np.float32)  # (19, 3)

    vb_t = vb.reshape(4, 128).T.copy().astype(np.float32)             # (128, 4)
    gam = np.array([[float(pam_gamma[0]), float(cam_gamma[0])]], np.float32)

    in_maps = []
    for c in range(NCORE):
        b_, s_ = divmod(c, S)
        r0 = s_ * RS - 1                               # first halo row
        # halo slices with zero pad
        fhs = np.zeros((4, 128, HR, W), dtype=bf16)
        ghs = np.zeros((4, 128, HR, W), dtype=bf16)
        qaug = np.zeros((65, NPIXH), dtype=np.float32)
        msk = np.zeros((HR, W), dtype=np.float32)
        lo, hi = max(r0, 0), min(r0 + HR, H)
        fhs[:, :, lo - r0:hi - r0, :] = f_full[b_, :, :, lo:hi, :].astype(bf16)
        ghs[:, :, lo - r0:hi - r0, :] = g_full[b_, :, :, lo:hi, :].astype(bf16)
        qaug[0:64].reshape(64, HR, W)[:, lo - r0:hi - r0, :] = \
            q_full[b_, :, lo:hi, :]
        msk[lo - r0:hi - r0, :] = 1.0
        mskb = np.broadcast_to(
            msk.reshape(1, NPIXH).astype(bf16), (128, NPIXH)
        ).copy()
        kaug = np.concatenate(
            [k_full[b_].reshape(64, N), np.ones((1, N), np.float32)], axis=0
        )
        in_maps.append({
            "KAUG": kaug, "QAUG": qaug,
            "KB16": kaug[0:64].astype(bf16), "QB16": qaug[0:64].astype(bf16),
            "VT2": vt_full[b_], "CEN": cen_full[b_],
            "FH": fhs, "GH": ghs,
            "W2T": w2t, "OSC": osc, "OSH": osh,
            "CLSW": clsw_t, "CLSB": clsb, "VB": vb_t, "GAM": gam,
            "MSKB": mskb,
        })
    return in_maps


def kernel(x, paW, pab, pa_g, pa_b, pa_m, pa_v,
           qW, qb, kW, kb, vW, vb, pam_gamma,
           paoW, paob, pao_g, pao_b, pao_m, pao_v, paclsW, paclsb,
           caW, cab, ca_g, ca_b, ca_m, ca_v, cam_gamma,
           caoW, caob, cao_g, cao_b, cao_m, cao_v, caclsW, caclsb,
           fW, fb, _profile=False):
    nc1, nc2 = _get_kernels()

    im1 = _prep_launch1(
        np.asarray(x), np.asarray(paW), np.asarray(pab),
        (np.asarray(pa_g), np.asarray(pa_b), np.asarray(pa_m), np.asarray(pa_v)),
        np.asarray(caW), np.asarray(cab),
        (np.asarray(ca_g), np.asarray(ca_b), np.asarray(ca_m), np.asarray(ca_v)),
        np.asarray(qW), np.asarray(qb), np.asarray(kW), np.asarray(kb),
        np.asarray(vW),
    )
    res1 = run_bass_kernel_spmd(nc1, im1, core_ids=list(range(NCORE)),
                                trace=_profile)
    t1 = res1.exec_time_ns

    im2 = _prep_launch2(
        res1.results,
        np.asarray(paoW), np.asarray(paob),
        (np.asarray(pao_g), np.asarray(pao_b), np.asarray(pao_m), np.asarray(pao_v)),
        np.asarray(caoW), np.asarray(caob),
        (np.asarray(cao_g), np.asarray(cao_b), np.asarray(cao_m), np.asarray(cao_v)),
        np.asarray(paclsW), np.asarray(paclsb),
        np.asarray(caclsW), np.asarray(caclsb),
        np.asarray(fW), np.asarray(fb),
        np.asarray(vb), np.asarray(pam_gamma), np.asarray(cam_gamma),
    )
    res2 = run_bass_kernel_spmd(nc2, im2, core_ids=list(range(NCORE)),
                                trace=_profile)
    t2 = res2.exec_time_ns

    fusion = np.zeros((B, NCLS, H, W), dtype=np.float32)
    pa_out = np.zeros((B, NCLS, H, W), dtype=np.float32)
    ca_out = np.zeros((B, NCLS, H, W), dtype=np.float32)
    for c in range(NCORE):
        b_, s_ = divmod(c, S)
        rows = slice(s_ * RS, (s_ + 1) * RS)
        o = res2.results[c]["OUT"]
        fusion[b_, :, rows, :] = o[0]
        pa_out[b_, :, rows, :] = o[1]
        ca_out[b_, :, rows, :] = o[2]

    if _profile:
        kernel.last_exec_ns = (t1, t2)
        kernel.last_results = (res1, res2)
    return (fusion, pa_out, ca_out)


# revision 29
# speedup vs baseline: 1.0146x; 1.0146x over previous
"""DANetHead (dual attention) Trainium2 kernel.

Full inputs in, full outputs out. Internally sharded over 8 NeuronCores:
core c -> batch b=c//4, row-slice s=c%4 (16 rows of the 64x64 image).
Two SPMD launches with host-side reshuffle between them:
  launch1: fused 3x3 conv (2048->1024: PA&CA branch convs together, f32r) +
           BN+ReLU, q/k 1x1 (f32r), v^T (bf16, transposed form), partial
           channel Gram matrix (f32r mm, fp32 accum; summed on host).
  launch2: PAM attention (transpose-free two-pass softmax), CAM channel
           attention, output convs (bf16), classifiers, fusion.

PAM softmax without transposing the attention matrix:
  pass1 (row-major e[i,j], bf16): per-query max M_i only. bf16 logit noise
        (+-8 of ~1.8e3) is harmless here: M only shifts the exps.
  pass2 (column-major e^T[j,i], fp32): contraction augmented to 65 rows
        (k' = [k; 1], q' = [q; -M]) so the PE emits e^T - M directly; ACT
        exps it straight into the [key, query] layout that the PA matmul
        consumes with v^T as lhsT -- no PE transposes, no DVE copies.
        Numerators need full fp32: f32r operand truncation on q/k (~+-30)
        gives +-3 logit noise, which scrambles the near-one-hot softmax.
  denominators: ones-column matmul over the same exp'd pT accumulated
        alongside the PA matmuls -- exactly consistent with numerators.
  normalization (gamma * mask / S) folds into the epilogue via a ones-lhsT
        broadcast matmul.

Precision elsewhere: f32r for the big convs / q/k projection / Gram
(validated: ca_out err 4.5e-3), bf16 after the softmaxes.
"""

import sys

sys.path.insert(0, "/opt/trn_rl_repo")

import numpy as np
import ml_dtypes

import concourse.bass as bass
import concourse.mybir as mybir
import concourse.tile as tile
from concourse import bacc
from concourse.bass_utils import run_bass_kernel_spmd
from concourse.masks import make_identity

BF16 = mybir.dt.bfloat16
F32 = mybir.dt.float32
F32R = mybir.dt.float32r
AF = mybir.ActivationFunctionType
ALU = mybir.AluOpType

B, CIN, H, W, NCLS = 2, 2048, 64, 64, 19
CI = 512          # inter channels
C8 = 64           # q/k channels
N = H * W         # 4096 pixels per image
NCORE = 8
S = 4             # row slices per batch
RS = H // S       # 16 rows per slice
HR = RS + 2       # 18 rows incl. halo
NPIX = RS * W     # 1024 pixels per slice
NPIXH = HR * W    # 1152 pixels incl. halo (the query set)
NT3 = NPIXH // 384  # 3 thirds of 384 queries
EPS = 1e-5

bf16 = ml_dtypes.bfloat16


# --------------------------------------------------------------------------
# launch 1: conv(2048 -> 1024, 3x3, f32r) + BN + ReLU ; qk(f32r) ; vT ; cen
# --------------------------------------------------------------------------

def build_launch1():
    nc = bacc.Bacc(None, target_bir_lowering=False)

    XP = nc.dram_tensor("XP", [128, 16, HR, W + 2], F32R, kind="ExternalInput")
    W1T = nc.dram_tensor("W1T", [8, 128, 16, 9, 128], F32R, kind="ExternalInput")
    FGSC = nc.dram_tensor("FGSC", [128, 8], F32, kind="ExternalInput")
    FGSH = nc.dram_tensor("FGSH", [128, 8], F32, kind="ExternalInput")
    QKWT = nc.dram_tensor("QKWT", [4, 128, 128], F32R, kind="ExternalInput")
    QKB = nc.dram_tensor("QKB", [128, 1], F32, kind="ExternalInput")
    VWT = nc.dram_tensor("VWT", [4, 128, 512], BF16, kind="ExternalInput")
    IDR = nc.dram_tensor("IDR", [128, 128], F32R, kind="ExternalInput")

    FG = nc.dram_tensor("FG", [8, 128, RS, W], F32R, kind="ExternalOutput")
    QK = nc.dram_tensor("QK", [128, NPIX], F32, kind="ExternalOutput")
    VT = nc.dram_tensor("VT", [8, 128, 512], BF16, kind="ExternalOutput")
    CENP = nc.dram_tensor("CENP", [4, 128, 512], F32, kind="ExternalOutput")

    with tile.TileContext(nc) as tc:
        with (
            tc.tile_pool(name="singles", bufs=1) as singles,
            tc.tile_pool(name="wpool", bufs=2) as wpool,
            tc.tile_pool(name="opool", bufs=2) as opool,
            tc.tile_pool(name="pspool", bufs=2, space="PSUM") as pspool,
        ):
            x_all = singles.tile([128, 16, HR, W + 2], F32R)

            # first conv chunk (x + weights) lands before anything else
            wv00 = wpool.tile([128, 4, 9, 128], F32R, tag="w")
            for t in range(4):
                nc.sync.dma_start(x_all[:, t], XP[:, t])
                nc.gpsimd.dma_start(wv00[:, t], W1T[4][:, t])

            fgsc = singles.tile([128, 8], F32)
            nc.sync.dma_start(fgsc[:], FGSC[:])
            fgsh = singles.tile([128, 8], F32)
            nc.sync.dma_start(fgsh[:], FGSH[:])
            qkwt = singles.tile([128, 4, 128], F32R)
            nc.sync.dma_start(qkwt[:], QKWT.ap().rearrange("t p c -> p t c"))
            qkb = singles.tile([128, 1], F32)
            nc.sync.dma_start(qkb[:], QKB[:])
            vwt = singles.tile([128, 4, 512], BF16)
            nc.sync.dma_start(vwt[:], VWT.ap().rearrange("t p c -> p t c"))
            identr = singles.tile([128, 128], F32R)
            nc.sync.dma_start(identr[:], IDR[:])

            # conv outputs: f32r resident (qk/cen matmuls) + bf16 copy (vt)
            fgout32 = singles.tile([128, 8, RS, W], F32R)
            fg_bf = singles.tile([128, 4, RS, W], BF16)
            gt32 = singles.tile([128, 8, 512], F32R)  # g^T per 128-px tile

            fgv32 = fgout32.rearrange("p t r c -> p t (r c)")
            fgv = fg_bf.rearrange("p t r c -> p t (r c)")

            def conv_cot(cot, first=False):
                acc2 = pspool.tile([128, 2, 8, W], F32, tag="conv")
                for ch in range(4):
                    if first and ch == 0:
                        wv = wv00
                    else:
                        wv = wpool.tile([128, 4, 9, 128], F32R, tag="w")
                        if first:
                            nc.sync.dma_start(
                                x_all[:, ch * 4:(ch + 1) * 4],
                                XP[:, ch * 4:(ch + 1) * 4],
                            )
                        nc.gpsimd.dma_start(wv[:], W1T[cot][:, ch * 4:(ch + 1) * 4])
                    for rb in range(2):
                        for cit4 in range(4):
                            for dd in range(9):
                                dy, dx = dd // 3, dd % 3
                                r0 = rb * 8 + dy
                                nc.tensor.matmul(
                                    acc2[:, rb],
                                    wv[:, cit4, dd, :],
                                    x_all[:, ch * 4 + cit4, r0:r0 + 8, dx:dx + W],
                                    start=(ch == 0 and cit4 == 0 and dd == 0),
                                    stop=(ch == 3 and cit4 == 3 and dd == 8),
                                )
                for rb in range(2):
                    nc.scalar.activation(
                        out=fgout32[:, cot, rb * 8:(rb + 1) * 8, :],
                        in_=acc2[:, rb],
                        func=AF.Relu,
                        bias=fgsh[:, cot:cot + 1],
                        scale=fgsc[:, cot:cot + 1],
                    )
                    if cot < 4:
                        nc.vector.tensor_copy(
                            fg_bf[:, cot, rb * 8:(rb + 1) * 8, :],
                            fgout32[:, cot, rb * 8:(rb + 1) * 8, :],
                        )
                    else:
                        # g^T transposes as soon as each 8-row block lands
                        ct = cot - 4
                        for nt in range(rb * 4, rb * 4 + 4):
                            tp = pspool.tile([128, 128], F32R, tag="small")
                            nc.tensor.transpose(
                                tp[:], fgv32[:, cot, nt * 128:(nt + 1) * 128],
                                identr[:],
                            )
                            nc.vector.tensor_copy(
                                gt32[:, nt, ct * 128:(ct + 1) * 128], tp[:]
                            )
                    nc.sync.dma_start(
                        FG[cot, :, rb * 8:(rb + 1) * 8, :],
                        fgout32[:, cot, rb * 8:(rb + 1) * 8, :],
                    )

            # ---- g branch convs first (transposes inlined per row-block);
            #      Gram halves then hide under the first f convs ----
            conv_cot(4, first=True)
            for ct in range(1, 4):
                conv_cot(4 + ct)

            cen_sb = opool.tile([128, 4, 512], F32, tag="cen_sb", bufs=1)

            def gram_half(half):
                cen_ps = pspool.tile([128, 2, 512], F32, tag="qkcen", bufs=1)
                for nt in range(8):
                    for ct2 in range(2):
                        ct = half * 2 + ct2
                        nc.tensor.matmul(
                            cen_ps[:, ct2, :],
                            gt32[:, nt, ct * 128:(ct + 1) * 128],
                            gt32[:, nt, :],
                            start=(nt == 0),
                            stop=(nt == 7),
                        )
                nc.vector.tensor_copy(
                    cen_sb[:, half * 2:(half + 1) * 2], cen_ps[:]
                )

            # ---- f branch convs, Gram interleaved ----
            conv_cot(0)
            gram_half(0)
            conv_cot(1)
            gram_half(1)
            nc.sync.dma_start(CENP.ap().rearrange("t p c -> p t c"), cen_sb[:])
            conv_cot(2)
            conv_cot(3)

            # ---- q/k : packed f32r matmul (q rows 0:64, k rows 64:128) ----
            qk_ps = pspool.tile([128, 2, 512], F32, tag="qkcen", bufs=1)
            for ck in range(2):
                for cit in range(4):
                    nc.tensor.matmul(
                        qk_ps[:, ck, :],
                        qkwt[:, cit, :],
                        fgv32[:, cit, ck * 512:(ck + 1) * 512],
                        start=(cit == 0),
                        stop=(cit == 3),
                    )
            qk_sb = opool.tile([128, NPIX], F32, tag="qk_sb")
            nc.vector.tensor_scalar(
                out=qk_sb[:], in0=qk_ps.rearrange("p a b -> p (a b)"),
                scalar1=qkb[:], scalar2=None, op0=ALU.add,
            )
            nc.sync.dma_start(QK[:], qk_sb[:])

            # ---- vT[n, c] (no bias: folded in launch2) ----
            for nt in range(8):
                vps = pspool.tile([128, 512], F32, tag="small")
                for cit in range(4):
                    nc.tensor.matmul(
                        vps[:],
                        fgv[:, cit, nt * 128:(nt + 1) * 128],
                        vwt[:, cit, :],
                        start=(cit == 0),
                        stop=(cit == 3),
                    )
                vt_sb = opool.tile([128, 512], BF16, tag="vt_sb")
                nc.vector.tensor_copy(vt_sb[:], vps[:])
                nc.sync.dma_start(VT[nt], vt_sb[:])

    nc.compile()
    return nc


# --------------------------------------------------------------------------
# launch 2: PAM (transpose-free) + CAM + output convs + classifiers + fusion
# --------------------------------------------------------------------------

def build_launch2():
    nc = bacc.Bacc(None, target_bir_lowering=False)

    KAUG = nc.dram_tensor("KAUG", [65, N], F32, kind="ExternalInput")
    QAUG = nc.dram_tensor("QAUG", [65, NPIXH], F32, kind="ExternalInput")
    KB16 = nc.dram_tensor("KB16", [64, N], BF16, kind="ExternalInput")
    QB16 = nc.dram_tensor("QB16", [64, NPIXH], BF16, kind="ExternalInput")
    VT2 = nc.dram_tensor("VT2", [32, 128, 512], BF16, kind="ExternalInput")
    CEN = nc.dram_tensor("CEN", [4, 128, 512], F32, kind="ExternalInput")
    FH = nc.dram_tensor("FH", [4, 128, HR, W], BF16, kind="ExternalInput")
    GH = nc.dram_tensor("GH", [4, 128, HR, W], BF16, kind="ExternalInput")
    W2T = nc.dram_tensor("W2T", [2, 4, 128, 4, 9, 128], BF16, kind="ExternalInput")
    OSC = nc.dram_tensor("OSC", [128, 8], F32, kind="ExternalInput")
    OSH = nc.dram_tensor("OSH", [128, 8], F32, kind="ExternalInput")
    CLSW = nc.dram_tensor("CLSW", [3, 4, 128, NCLS], BF16, kind="ExternalInput")
    CLSB = nc.dram_tensor("CLSB", [NCLS, 3], F32, kind="ExternalInput")
    VB = nc.dram_tensor("VB", [128, 4], F32, kind="ExternalInput")
    GAM = nc.dram_tensor("GAM", [1, 2], F32, kind="ExternalInput")
    MSKB = nc.dram_tensor("MSKB", [128, NPIXH], BF16, kind="ExternalInput")

    OUT = nc.dram_tensor("OUT", [3, NCLS, RS, W], F32, kind="ExternalOutput")

    with tile.TileContext(nc) as tc:
        with (
            tc.tile_pool(name="singles", bufs=1) as singles,
            tc.tile_pool(name="w2p", bufs=2) as w2p,
            tc.tile_pool(name="work", bufs=2) as work,
            tc.tile_pool(name="cols", bufs=4) as cols,
            tc.tile_pool(name="pspool", bufs=2, space="PSUM") as pspool,
        ):
            # ---- input DMAs, roughly in order of first use ----
            qb16 = singles.tile([64, NPIXH], BF16)
            nc.sync.dma_start(qb16[:], QB16[:])
            kb16 = singles.tile([64, N], BF16)
            nc.sync.dma_start(kb16[:], KB16[:])
            cen = singles.tile([128, 4, 512], F32)
            nc.sync.dma_start(cen[:], CEN.ap().rearrange("t p c -> p t c"))
            gh = singles.tile([128, 4, HR, W], BF16)
            nc.sync.dma_start(gh[:], GH.ap().rearrange("t p r c -> p t r c"))
            gam_pa = singles.tile([128, 1], F32)
            nc.sync.dma_start(
                gam_pa[:],
                bass.AP(tensor=GAM.ap().tensor, offset=0, ap=[[0, 128], [1, 1]]),
            )
            gam_ca = singles.tile([128, 1], F32)
            nc.sync.dma_start(
                gam_ca[:],
                bass.AP(tensor=GAM.ap().tensor, offset=1, ap=[[0, 128], [1, 1]]),
            )
            vb = singles.tile([128, 4], F32)
            nc.sync.dma_start(vb[:], VB[:])
            osc = singles.tile([128, 8], F32)
            nc.sync.dma_start(osc[:], OSC[:])
            osh = singles.tile([128, 8], F32)
            nc.sync.dma_start(osh[:], OSH[:])
            kaug = singles.tile([65, N], F32)
            nc.sync.dma_start(kaug[:], KAUG[:])
            qaug = singles.tile([65, NPIXH], F32)
            nc.sync.dma_start(qaug[0:64, :], QAUG[0:64, :])
            mskb = singles.tile([128, NPIXH], BF16)
            nc.sync.dma_start(mskb[:], MSKB[:])
            fh = singles.tile([128, 4, HR, W], BF16)
            nc.sync.dma_start(fh[:], FH.ap().rearrange("t p r c -> p t r c"))
            vt = singles.tile([128, 32, 512], BF16)
            nc.sync.dma_start(vt[:], VT2.ap().rearrange("n p c -> p n c"))
            clsw = singles.tile([128, 3, 4, NCLS], BF16)
            nc.sync.dma_start(clsw[:], CLSW.ap().rearrange("w t p c -> p w t c"))
            ones1 = singles.tile([1, 128], F32)
            nc.sync.dma_start(ones1[:], KAUG[64:65, 0:128])
            clsb = singles.tile([NCLS, 3], F32)
            nc.sync.dma_start(clsb[:], CLSB[:])

            identf = singles.tile([128, 128], F32)
            make_identity(nc, identf[:])
            identb = singles.tile([128, 128], BF16)
            make_identity(nc, identb[:])
            onesj = singles.tile([128, 1], BF16)
            nc.vector.memset(onesj[:], 1.0)

            ghv = gh.rearrange("p t r c -> p t (r c)")
            fhv = fh.rearrange("p t r c -> p t (r c)")

            gvb = singles.tile([128, 4], F32)
            nc.vector.tensor_scalar(
                out=gvb[:], in0=vb[:], scalar1=gam_pa[:], scalar2=None,
                op0=ALU.mult,
            )

            negm9 = singles.tile([128, 9], F32)
            feat_bf = singles.tile([128, 2, 4, RS, W], BF16)
            pT = singles.tile([128, 32, 384], BF16)
            pabuf = singles.tile([128, 4, HR, W + 2], BF16)
            cabuf = singles.tile([128, 4, HR, W + 2], BF16)
            nc.vector.memset(cabuf[:], 0.0)

            # ---- PAM pass 1: bf16 row-major energies, per-query max ----
            def pass1_it(it):
                nmx8 = cols.tile([128, 8], F32, tag="nmx8")
                for jc in range(8):
                    eps = pspool.tile([128, 512], F32, tag="e")
                    nc.tensor.matmul(
                        eps[:],
                        qb16[:, it * 128:(it + 1) * 128],
                        kb16[:, jc * 512:(jc + 1) * 512],
                        start=True, stop=True,
                    )
                    nc.vector.tensor_reduce(
                        out=nmx8[:, jc:jc + 1], in_=eps[:], op=ALU.max,
                        axis=mybir.AxisListType.X, negate=True,
                    )
                nc.vector.tensor_reduce(
                    out=negm9[:, it:it + 1], in_=nmx8[:], op=ALU.min,
                    axis=mybir.AxisListType.X,
                )

            def third_negm(k):
                # -M into qaug row 64 (3 single-partition DMAs)
                tpn = pspool.tile([3, 128], F32, tag="e")
                nc.tensor.transpose(tpn[:], negm9[:, 3 * k:3 * k + 3], identf[:])
                rowr = work.tile([3, 128], F32, tag="rowr")
                nc.vector.tensor_copy(rowr[:], tpn[:])
                for a in range(3):
                    nc.sync.dma_start(
                        qaug[64:65, 384 * k + 128 * a:384 * k + 128 * (a + 1)],
                        rowr[a:a + 1, :],
                    )

            def ca_branch():
                E_sb = singles.tile([128, 4, 512], BF16)
                Scol = singles.tile([128, 4], F32)
                for ct in range(4):
                    mn = cols.tile([128, 1], F32, tag="camn")
                    nc.vector.tensor_reduce(
                        out=mn[:], in_=cen[:, ct, :], op=ALU.min,
                        axis=mybir.AxisListType.X,
                    )
                    nc.scalar.activation(
                        out=E_sb[:, ct, :], in_=cen[:, ct, :], func=AF.Exp,
                        bias=mn[:], scale=-1.0, accum_out=Scol[:, ct:ct + 1],
                    )
                grS = singles.tile([128, 4], F32)
                nc.vector.reciprocal(grS[:], Scol[:])
                nc.vector.tensor_scalar(
                    out=grS[:], in0=grS[:], scalar1=gam_ca[:], scalar2=None,
                    op0=ALU.mult,
                )
                ET = singles.tile([128, 4, 512], BF16)
                for ct in range(4):
                    for dt in range(4):
                        tp = pspool.tile([128, 128], BF16, tag="e")
                        nc.tensor.transpose(
                            tp[:], E_sb[:, ct, dt * 128:(dt + 1) * 128],
                            identb[:],
                        )
                        nc.vector.tensor_copy(
                            ET[:, dt, ct * 128:(ct + 1) * 128], tp[:]
                        )
                for ck in range(3):
                    px0 = ck * 384
                    ca_ps = pspool.tile([128, 4, 512], F32, tag="pa", bufs=1)
                    for ct in range(4):
                        for dt in range(4):
                            nc.tensor.matmul(
                                ca_ps[:, ct, :384],
                                ET[:, dt, ct * 128:(ct + 1) * 128],
                                ghv[:, dt, px0:px0 + 384],
                                start=(dt == 0),
                                stop=(dt == 3),
                            )
                    for ct in range(4):
                        tmp = work.tile([128, 384], F32, tag="catmp")
                        nc.vector.tensor_scalar(
                            out=tmp[:], in0=ca_ps[:, ct, :384],
                            scalar1=grS[:, ct:ct + 1], scalar2=None,
                            op0=ALU.mult,
                        )
                        nc.vector.tensor_add(
                            cabuf[:, ct, ck * 6:(ck + 1) * 6, 1:1 + W],
                            tmp.rearrange("p (r c) -> p r c", c=W),
                            ghv[:, ct, px0:px0 + 384]
                            .rearrange("p (r c) -> p r c", c=W),
                        )

            def w2_load(br, cot):
                w2v = w2p.tile([128, 4, 9, 128], BF16, tag="w2")
                nc.sync.dma_start(w2v[:], W2T[br, cot])
                return w2v

            def conv_group(br, buf, cot, rb, w2v):
                acc = pspool.tile([128, 8, W], F32, tag="cv")
                nmm = 0
                for cit in range(4):
                    wq = w2v[:, cit]
                    for dd in range(9):
                        dy, dx = dd // 3, dd % 3
                        r0 = rb * 8 + dy
                        nc.tensor.matmul(
                            acc[:],
                            wq[:, dd, :],
                            buf[:, cit, r0:r0 + 8, dx:dx + W],
                            start=(nmm == 0),
                            stop=(nmm == 35),
                        )
                        nmm += 1
                nc.scalar.activation(
                    out=feat_bf[:, br, cot, rb * 8:(rb + 1) * 8, :],
                    in_=acc[:],
                    func=AF.Relu,
                    bias=osh[:, br * 4 + cot:br * 4 + cot + 1],
                    scale=osc[:, br * 4 + cot:br * 4 + cot + 1],
                )

            featv = feat_bf.rearrange("p b t r c -> p b t (r c)")

            def classifier_ck(which, ck):
                # which 0: fusion (paf + caf through fW), 1: pa, 2: ca
                # ck 0 covers rows 0..7 (rb0 features), ck 1 rows 8..15
                sl = slice(ck * 512, (ck + 1) * 512)
                cls_ps = pspool.tile([NCLS, 512], F32, tag="cv")
                if which == 0:
                    nmm = 0
                    for br in range(2):
                        for cit in range(4):
                            nc.tensor.matmul(
                                cls_ps[:],
                                clsw[:, 0, cit, :],
                                featv[:, br, cit, sl],
                                start=(nmm == 0), stop=(nmm == 7),
                            )
                            nmm += 1
                else:
                    br = which - 1
                    for cit in range(4):
                        nc.tensor.matmul(
                            cls_ps[:],
                            clsw[:, which, cit, :],
                            featv[:, br, cit, sl],
                            start=(cit == 0), stop=(cit == 3),
                        )
                out_sb = work.tile([NCLS, 512], F32, tag="out_sb")
                nc.vector.tensor_scalar(
                    out=out_sb[:], in0=cls_ps[:],
                    scalar1=clsb[:, which:which + 1], scalar2=None,
                    op0=ALU.add,
                )
                nc.sync.dma_start(
                    OUT[which, :, ck * 8:(ck + 1) * 8, :]
                    .rearrange("p r c -> p (r c)"),
                    out_sb[:],
                )

            # ---- pass1 its interleaved with CA branch + cao conv ----
            cao_groups = [(cot, rb) for cot in range(4) for rb in range(2)]
            w2v_live = {}

            def cao_group(g):
                cot, rb = cao_groups[g]
                if rb == 0:
                    w2v_live[cot] = w2_load(1, cot)
                conv_group(1, cabuf, cot, rb, w2v_live[cot])

            pass1_it(0)
            pass1_it(1)
            ca_branch()
            pass1_it(2)
            third_negm(0)
            for it in range(3, 9):
                pass1_it(it)
                if it == 5:
                    third_negm(1)
                elif it == 8:
                    third_negm(2)
                cao_group(it - 3)

            nc.vector.memset(pabuf[:], 0.0)

            # FM = (f + gamma*vb) * mask  (residual term of the PA epilogue)
            FM = singles.tile([128, 4, NPIXH], BF16)
            for ct in range(4):
                nc.vector.tensor_scalar(
                    out=FM[:, ct, :], in0=fhv[:, ct, :],
                    scalar1=gvb[:, ct:ct + 1], scalar2=None, op0=ALU.add,
                )
                nc.vector.tensor_mul(FM[:, ct, :], FM[:, ct, :], mskb[:])

            # ---- PAM pass 2 + PA accumulation, per third ----
            def eT_exp(k, jt):
                eT = pspool.tile([128, 384], F32, tag="e")
                nc.tensor.matmul(
                    eT[:],
                    kaug[:, jt * 128:(jt + 1) * 128],
                    qaug[:, 384 * k:384 * (k + 1)],
                    start=True, stop=True,
                )
                nc.scalar.activation(
                    out=pT[:, jt, :], in_=eT[:], func=AF.Exp,
                    bias=0.0, scale=1.0,
                )

            for k in range(NT3):
                q_sl = slice(384 * k, 384 * (k + 1))
                if k == 0:
                    for jt in range(16):
                        eT_exp(0, jt)
                    cao_group(6)
                    for jt in range(16, 32):
                        eT_exp(0, jt)
                    cao_group(7)
                    classifier_ck(2, 0)     # ca head: only needs cao output
                    classifier_ck(2, 1)
                    w2v_live[0] = w2_load(0, 0)
                    w2v_live[1] = w2_load(0, 1)
                pa_ps = pspool.tile([128, 4, 512], F32, tag="pa", bufs=1)
                dn = pspool.tile([1, 384], F32, tag="cv")
                for jt in range(32):
                    for ct in range(4):
                        nc.tensor.matmul(
                            pa_ps[:, ct, :384],
                            vt[:, jt, ct * 128:(ct + 1) * 128],
                            pT[:, jt, :],
                            start=(jt == 0),
                            stop=(jt == 31),
                        )
                    nc.tensor.matmul(
                        dn[:], onesj[:], pT[:, jt, :],
                        start=(jt == 0), stop=(jt == 31),
                    )
                    if k + 1 < NT3:
                        eT_exp(k + 1, jt)
                # R = gamma * mask / S, broadcast over channel partitions
                R3r = work.tile([1, 384], F32, tag="R3r")
                nc.vector.reciprocal(R3r[:], dn[:])
                nc.vector.tensor_scalar(
                    out=R3r[:], in0=R3r[:], scalar1=gam_pa[0:1, :],
                    scalar2=None, op0=ALU.mult,
                )
                nc.vector.tensor_mul(R3r[:], R3r[:], mskb[0:1, q_sl])
                rb_ps = pspool.tile([128, 384], F32, tag="e")
                nc.tensor.matmul(rb_ps[:], ones1[:], R3r[:],
                                 start=True, stop=True)
                Rbm = work.tile([128, 384], F32, tag="Rbm")
                nc.vector.tensor_copy(Rbm[:], rb_ps[:])
                # epilogue: pabuf = pa * R + FM
                for ct in range(4):
                    tmp2 = work.tile([128, 384], F32, tag="patmp")
                    nc.vector.tensor_mul(tmp2[:], pa_ps[:, ct, :384], Rbm[:])
                    nc.vector.tensor_add(
                        pabuf[:, ct, k * 6:(k + 1) * 6, 1:1 + W],
                        tmp2.rearrange("p (r c) -> p r c", c=W),
                        FM[:, ct, q_sl].rearrange("p (r c) -> p r c", c=W),
                    )
                if k == 1:
                    # pao rb0: needs pabuf rows 0..9 (thirds 0 and 1)
                    conv_group(0, pabuf, 0, 0, w2v_live[0])
                    w2v_live[2] = w2_load(0, 2)
                    conv_group(0, pabuf, 1, 0, w2v_live[1])
                    w2v_live[3] = w2_load(0, 3)
                    conv_group(0, pabuf, 2, 0, w2v_live[2])
                    conv_group(0, pabuf, 3, 0, w2v_live[3])
                    classifier_ck(0, 0)     # rows 0..7 ready
                    classifier_ck(1, 0)
                elif k == 2:
                    w2v_live[0] = w2_load(0, 0)
                    w2v_live[1] = w2_load(0, 1)

            # pao rb1 (pabuf rows 8..17)
            conv_group(0, pabuf, 0, 1, w2v_live[0])
            w2v_live[2] = w2_load(0, 2)
            conv_group(0, pabuf, 1, 1, w2v_live[1])
            w2v_live[3] = w2_load(0, 3)
            conv_group(0, pabuf, 2, 1, w2v_live[2])
            conv_group(0, pabuf, 3, 1, w2v_live[3])

            classifier_ck(0, 1)
            classifier_ck(1, 1)

    nc.compile()
    return nc


# --------------------------------------------------------------------------
# host-side preparation and glue
# --------------------------------------------------------------------------

_CACHE = {}


def _get_kernels():
    if "nc1" not in _CACHE:
        _CACHE["nc1"] = build_launch1()
        _CACHE["nc2"] = build_launch2()
    return _CACHE["nc1"], _CACHE["nc2"]


def _fold_bn(g, b, m, v, conv_b):
    scale = g / np.sqrt(v + EPS)
    shift = (conv_b - m) * scale + b
    return scale.astype(np.float32), shift.astype(np.float32)


def _prep_launch1(x, paW, pab, pa_bn, caW, cab, ca_bn, qW, qb, kW, kb, vW):
    """Build the 8 per-core input maps for launch 1."""
    W1 = np.concatenate([paW, caW], axis=0)            # (1024, 2048, 3, 3)
    w1t = np.ascontiguousarray(
        np.transpose(W1.reshape(8, 128, 16, 128, 3, 3), (0, 3, 2, 4, 5, 1))
    ).reshape(8, 128, 16, 9, 128).astype(np.float32)

    sc_f, sh_f = _fold_bn(*pa_bn, pab)
    sc_g, sh_g = _fold_bn(*ca_bn, cab)
    fgsc = np.concatenate([sc_f, sc_g]).reshape(8, 128).T.copy()   # (128, 8)
    fgsh = np.concatenate([sh_f, sh_g]).reshape(8, 128).T.copy()

    qkW = np.concatenate([qW[:, :, 0, 0], kW[:, :, 0, 0]], axis=0)  # (128, 512)
    qkwt = np.ascontiguousarray(
        qkW.T.reshape(4, 128, 128)
    ).astype(np.float32)                               # [cit, ci, co]
    qkb_ = np.concatenate([qb, kb]).reshape(128, 1).astype(np.float32)
    vwt = np.ascontiguousarray(
        vW[:, :, 0, 0].T.reshape(4, 128, 512)
    ).astype(bf16)
    idr = np.eye(128, dtype=np.float32)

    # padded input slices, pre-transposed to partition-major layout
    xpad = np.zeros((B, CIN, H + 2, W + 2), dtype=np.float32)
    xpad[:, :, 1:H + 1, 1:W + 1] = x.astype(np.float32)

    in_maps = []
    for c in range(NCORE):
        b_, s_ = divmod(c, S)
        rows = slice(s_ * RS, s_ * RS + HR)            # in padded coords
        xp = np.ascontiguousarray(
            xpad[b_, :, rows, :].reshape(16, 128, HR, W + 2)
            .transpose(1, 0, 2, 3)
        )
        in_maps.append({
            "XP": xp, "W1T": w1t, "FGSC": fgsc, "FGSH": fgsh,
            "QKWT": qkwt, "QKB": qkb_, "VWT": vwt, "IDR": idr,
        })
    return in_maps


def _prep_launch2(r1, paoW, paob, pao_bn, caoW, caob, cao_bn,
                  paclsW, paclsb, caclsW, caclsb, fW, fb,
                  vb, pam_gamma, cam_gamma):
    """Reshuffle launch-1 outputs and build launch-2 input maps."""
    f_full = np.zeros((B, 4, 128, H, W), dtype=np.float32)
    g_full = np.zeros((B, 4, 128, H, W), dtype=np.float32)
    q_full = np.zeros((B, 64, H, W), dtype=np.float32)
    k_full = np.zeros((B, 64, H, W), dtype=np.float32)
    vt_full = np.zeros((B, 32, 128, 512), dtype=bf16)
    cen_full = np.zeros((B, 4, 128, 512), dtype=np.float32)
    for c in range(NCORE):
        b_, s_ = divmod(c, S)
        r = r1[c]
        rows = slice(s_ * RS, (s_ + 1) * RS)
        f_full[b_, :, :, rows, :] = r["FG"][0:4]
        g_full[b_, :, :, rows, :] = r["FG"][4:8]
        qk = r["QK"].reshape(128, RS, W)
        q_full[b_, :, rows, :] = qk[0:64]
        k_full[b_, :, rows, :] = qk[64:128]
        vt_full[b_, s_ * 8:(s_ + 1) * 8] = r["VT"]
        cen_full[b_] += r["CENP"]

    w2 = np.stack([paoW, caoW])                        # (2, 512, 512, 3, 3)
    w2t = np.ascontiguousarray(
        np.transpose(w2.reshape(2, 4, 128, 4, 128, 3, 3), (0, 1, 4, 3, 5, 6, 2))
    ).reshape(2, 4, 128, 4, 9, 128).astype(bf16)

    sc_p, sh_p = _fold_bn(*pao_bn, paob)
    sc_c, sh_c = _fold_bn(*cao_bn, caob)
    osc = np.concatenate([sc_p, sc_c]).reshape(8, 128).T.copy()
    osh = np.concatenate([sh_p, sh_c]).reshape(8, 128).T.copy()

    clsw = np.stack([
        fW[:, :, 0, 0], paclsW[:, :, 0, 0], caclsW[:, :, 0, 0]
    ])                                                 # (3, 19, 512)
    clsw_t = np.ascontiguousarray(
        np.transpose(clsw.reshape(3, NCLS, 4, 128), (0, 2, 3, 1))
    ).astype(bf16)                                     # (3, 4, 128, 19)
    clsb = np.stack([fb, paclsb, caclsb], axis=1).astype(np.float32)  # (19, 3)

    vb_t = vb.reshape(4, 128).T.copy().astype(np.float32)             # (128, 4)
    gam = np.array([[float(pam_gamma[0]), float(cam_gamma[0])]], np.float32)

    in_maps = []
    for c in range(NCORE):
        b_, s_ = divmod(c, S)
        r0 = s_ * RS - 1                               # first halo row
        # halo slices with zero pad
        fhs = np.zeros((4, 128, HR, W), dtype=bf16)
        ghs = np.zeros((4, 128, HR, W), dtype=bf16)
        qaug = np.zeros((65, NPIXH), dtype=np.float32)
        msk = np.zeros((HR, W), dtype=np.float32)
        lo, hi = max(r0, 0), min(r0 + HR, H)
        fhs[:, :, lo - r0:hi - r0, :] = f_full[b_, :, :, lo:hi, :].astype(bf16)
        ghs[:, :, lo - r0:hi - r0, :] = g_full[b_, :, :, lo:hi, :].astype(bf16)
        qaug[0:64].reshape(64, HR, W)[:, lo - r0:hi - r0, :] = \
            q_full[b_, :, lo:hi, :]
        msk[lo - r0:hi - r0, :] = 1.0
        mskb = np.broadcast_to(
            msk.reshape(1, NPIXH).astype(bf16), (128, NPIXH)
        ).copy()
        kaug = np.concatenate(
            [k_full[b_].reshape(64, N), np.ones((1, N), np.float32)], axis=0
        )
        in_maps.append({
            "KAUG": kaug, "QAUG": qaug,
            "KB16": kaug[0:64].astype(bf16), "QB16": qaug[0:64].astype(bf16),
            "VT2": vt_full[b_], "CEN": cen_full[b_],
            "FH": fhs, "GH": ghs,
            "W2T": w2t, "OSC": osc, "OSH": osh,
            "CLSW": clsw_t, "CLSB": clsb, "VB": vb_t, "GAM": gam,
            "MSKB": mskb,
        })
    return in_maps


def kernel(x, paW, pab, pa_g, pa_b, pa_m, pa_v,
           qW, qb, kW, kb, vW, vb, pam_gamma,
           paoW, paob, pao_g, pao_b, pao_m, pao_v, paclsW, paclsb,
           caW, cab, ca_g, ca_b, ca_m, ca_v, cam_gamma,
           caoW, caob, cao_g, cao_b, cao_m, cao_v, caclsW, caclsb,
           fW, fb, _profile=False):
    nc1, nc2 = _get_kernels()

    im1 = _prep_launch1(
        np.asarray(x), np.asarray(paW), np.asarray(pab),
        (np.asarray(pa_g), np.asarray(pa_b), np.asarray(pa_m), np.asarray(pa_v)),
        np.asarray(caW), np.asarray(cab),
        (np.asarray(ca_g), np.asarray(ca_b), np.asarray(ca_m), np.asarray(ca_v)),
        np.asarray(qW), np.asarray(qb), np.asarray(kW), np.asarray(kb),
        np.asarray(vW),
    )
    res1 = run_bass_kernel_spmd(nc1, im1, core_ids=list(range(NCORE)),
                                trace=_profile)
    t1 = res1.exec_time_ns

    im2 = _prep_launch2(
        res1.results,
        np.asarray(paoW), np.asarray(paob),
        (np.asarray(pao_g), np.asarray(pao_b), np.asarray(pao_m), np.asarray(pao_v)),
        np.asarray(caoW), np.asarray(caob),
        (np.asarray(cao_g), np.asarray(cao_b), np.asarray(cao_m), np.asarray(cao_v)),
        np.asarray(paclsW), np.asarray(paclsb),
        np.asarray(caclsW), np.asarray(caclsb),
        np.asarray(fW), np.asarray(fb),
        np.asarray(vb), np.asarray(pam_gamma), np.asarray(cam_gamma),
    )
    res2 = run_bass_kernel_spmd(nc2, im2, core_ids=list(range(NCORE)),
                                trace=_profile)
    t2 = res2.exec_time_ns

    fusion = np.zeros((B, NCLS, H, W), dtype=np.float32)
    pa_out = np.zeros((B, NCLS, H, W), dtype=np.float32)
    ca_out = np.zeros((B, NCLS, H, W), dtype=np.float32)
    for c in range(NCORE):
        b_, s_ = divmod(c, S)
        rows = slice(s_ * RS, (s_ + 1) * RS)
        o = res2.results[c]["OUT"]
        fusion[b_, :, rows, :] = o[0]
        pa_out[b_, :, rows, :] = o[1]
        ca_out[b_, :, rows, :] = o[2]

    if _profile:
        kernel.last_exec_ns = (t1, t2)
        kernel.last_results = (res1, res2)
    return (fusion, pa_out, ca_out)


# revision 30
# speedup vs baseline: 1.0169x; 1.0023x over previous
"""DANetHead (dual attention) Trainium2 kernel.

Full inputs in, full outputs out. Internally sharded over 8 NeuronCores:
core c -> batch b=c//4, row-slice s=c%4 (16 rows of the 64x64 image).
Two SPMD launches with host-side reshuffle between them:
  launch1: fused 3x3 conv (2048->1024: PA&CA branch convs together, f32r) +
           BN+ReLU, q/k 1x1 (f32r), v^T (bf16, transposed form), partial
           channel Gram matrix (f32r mm, fp32 accum; summed on host).
  launch2: PAM attention (transpose-free two-pass softmax), CAM channel
           attention, output convs (bf16), classifiers, fusion.

PAM softmax without transposing the attention matrix:
  pass1 (row-major e[i,j], bf16): per-query max M_i only. bf16 logit noise
        (+-8 of ~1.8e3) is harmless here: M only shifts the exps.
  pass2 (column-major e^T[j,i], fp32): contraction augmented to 65 rows
        (k' = [k; 1], q' = [q; -M]) so the PE emits e^T - M directly; ACT
        exps it straight into the [key, query] layout that the PA matmul
        consumes with v^T as lhsT -- no PE transposes, no DVE copies.
        Numerators need full fp32: f32r operand truncation on q/k (~+-30)
        gives +-3 logit noise, which scrambles the near-one-hot softmax.
  denominators: ones-column matmul over the same exp'd pT accumulated
        alongside the PA matmuls -- exactly consistent with numerators.
  normalization (gamma * mask / S) folds into the epilogue via a ones-lhsT
        broadcast matmul.

Precision elsewhere: f32r for the big convs / q/k projection / Gram
(validated: ca_out err 4.5e-3), bf16 after the softmaxes.
"""

import sys

sys.path.insert(0, "/opt/trn_rl_repo")

import numpy as np
import ml_dtypes

import concourse.bass as bass
import concourse.mybir as mybir
import concourse.tile as tile
from concourse import bacc
from concourse.bass_utils import run_bass_kernel_spmd
from concourse.masks import make_identity

BF16 = mybir.dt.bfloat16
F32 = mybir.dt.float32
F32R = mybir.dt.float32r
AF = mybir.ActivationFunctionType
ALU = mybir.AluOpType

B, CIN, H, W, NCLS = 2, 2048, 64, 64, 19
CI = 512          # inter channels
C8 = 64           # q/k channels
N = H * W         # 4096 pixels per image
NCORE = 8
S = 4             # row slices per batch
RS = H // S       # 16 rows per slice
HR = RS + 2       # 18 rows incl. halo
NPIX = RS * W     # 1024 pixels per slice
NPIXH = HR * W    # 1152 pixels incl. halo (the query set)
NT3 = NPIXH // 384  # 3 thirds of 384 queries
EPS = 1e-5

bf16 = ml_dtypes.bfloat16


# --------------------------------------------------------------------------
# launch 1: conv(2048 -> 1024, 3x3, f32r) + BN + ReLU ; qk(f32r) ; vT ; cen
# --------------------------------------------------------------------------

def build_launch1():
    nc = bacc.Bacc(None, target_bir_lowering=False)

    XP = nc.dram_tensor("XP", [128, 16, HR, W + 2], F32R, kind="ExternalInput")
    W1T = nc.dram_tensor("W1T", [8, 128, 16, 9, 128], F32R, kind="ExternalInput")
    FGSC = nc.dram_tensor("FGSC", [128, 8], F32, kind="ExternalInput")
    FGSH = nc.dram_tensor("FGSH", [128, 8], F32, kind="ExternalInput")
    QKWT = nc.dram_tensor("QKWT", [4, 128, 128], F32R, kind="ExternalInput")
    QKB = nc.dram_tensor("QKB", [128, 1], F32, kind="ExternalInput")
    VWT = nc.dram_tensor("VWT", [4, 128, 512], BF16, kind="ExternalInput")
    IDR = nc.dram_tensor("IDR", [128, 128], F32R, kind="ExternalInput")

    FG = nc.dram_tensor("FG", [8, 128, RS, W], F32R, kind="ExternalOutput")
    QK = nc.dram_tensor("QK", [128, NPIX], F32, kind="ExternalOutput")
    VT = nc.dram_tensor("VT", [8, 128, 512], BF16, kind="ExternalOutput")
    CENP = nc.dram_tensor("CENP", [4, 128, 512], F32, kind="ExternalOutput")

    with tile.TileContext(nc) as tc:
        with (
            tc.tile_pool(name="singles", bufs=1) as singles,
            tc.tile_pool(name="wpool", bufs=2) as wpool,
            tc.tile_pool(name="opool", bufs=2) as opool,
            tc.tile_pool(name="pspool", bufs=2, space="PSUM") as pspool,
        ):
            x_all = singles.tile([128, 16, HR, W + 2], F32R)

            # first conv chunk (x + weights) lands before anything else
            wv00 = wpool.tile([128, 4, 9, 128], F32R, tag="w")
            for t in range(4):
                nc.sync.dma_start(x_all[:, t], XP[:, t])
                nc.gpsimd.dma_start(wv00[:, t], W1T[4][:, t])

            fgsc = singles.tile([128, 8], F32)
            nc.sync.dma_start(fgsc[:], FGSC[:])
            fgsh = singles.tile([128, 8], F32)
            nc.sync.dma_start(fgsh[:], FGSH[:])
            qkwt = singles.tile([128, 4, 128], F32R)
            nc.sync.dma_start(qkwt[:], QKWT.ap().rearrange("t p c -> p t c"))
            qkb = singles.tile([128, 1], F32)
            nc.sync.dma_start(qkb[:], QKB[:])
            vwt = singles.tile([128, 4, 512], BF16)
            nc.sync.dma_start(vwt[:], VWT.ap().rearrange("t p c -> p t c"))
            identr = singles.tile([128, 128], F32R)
            nc.sync.dma_start(identr[:], IDR[:])

            # conv outputs: f32r resident (qk/cen matmuls) + bf16 copy (vt)
            fgout32 = singles.tile([128, 8, RS, W], F32R)
            fg_bf = singles.tile([128, 4, RS, W], BF16)
            gt32 = singles.tile([128, 8, 512], F32R)  # g^T per 128-px tile

            fgv32 = fgout32.rearrange("p t r c -> p t (r c)")
            fgv = fg_bf.rearrange("p t r c -> p t (r c)")

            def conv_cot(cot, first=False):
                acc2 = pspool.tile([128, 2, 8, W], F32, tag="conv")
                for ch in range(4):
                    if first and ch == 0:
                        wv = wv00
                    else:
                        wv = wpool.tile([128, 4, 9, 128], F32R, tag="w")
                        if first:
                            nc.sync.dma_start(
                                x_all[:, ch * 4:(ch + 1) * 4],
                                XP[:, ch * 4:(ch + 1) * 4],
                            )
                        nc.gpsimd.dma_start(wv[:], W1T[cot][:, ch * 4:(ch + 1) * 4])
                    for rb in range(2):
                        for cit4 in range(4):
                            for dd in range(9):
                                dy, dx = dd // 3, dd % 3
                                r0 = rb * 8 + dy
                                nc.tensor.matmul(
                                    acc2[:, rb],
                                    wv[:, cit4, dd, :],
                                    x_all[:, ch * 4 + cit4, r0:r0 + 8, dx:dx + W],
                                    start=(ch == 0 and cit4 == 0 and dd == 0),
                                    stop=(ch == 3 and cit4 == 3 and dd == 8),
                                )
                for rb in range(2):
                    nc.scalar.activation(
                        out=fgout32[:, cot, rb * 8:(rb + 1) * 8, :],
                        in_=acc2[:, rb],
                        func=AF.Relu,
                        bias=fgsh[:, cot:cot + 1],
                        scale=fgsc[:, cot:cot + 1],
                    )
                    if cot < 4:
                        nc.vector.tensor_copy(
                            fg_bf[:, cot, rb * 8:(rb + 1) * 8, :],
                            fgout32[:, cot, rb * 8:(rb + 1) * 8, :],
                        )
                    else:
                        # g^T transposes as soon as each 8-row block lands
                        ct = cot - 4
                        for nt in range(rb * 4, rb * 4 + 4):
                            tp = pspool.tile([128, 128], F32R, tag="small")
                            nc.tensor.transpose(
                                tp[:], fgv32[:, cot, nt * 128:(nt + 1) * 128],
                                identr[:],
                            )
                            nc.vector.tensor_copy(
                                gt32[:, nt, ct * 128:(ct + 1) * 128], tp[:]
                            )
                    nc.sync.dma_start(
                        FG[cot, :, rb * 8:(rb + 1) * 8, :],
                        fgout32[:, cot, rb * 8:(rb + 1) * 8, :],
                    )

            # ---- g branch convs first (transposes inlined per row-block);
            #      Gram halves then hide under the first f convs ----
            conv_cot(4, first=True)
            for ct in range(1, 4):
                conv_cot(4 + ct)

            cen_sb = opool.tile([128, 4, 512], F32, tag="cen_sb", bufs=1)

            def gram_half(half):
                cen_ps = pspool.tile([128, 2, 512], F32, tag="qkcen", bufs=1)
                for nt in range(8):
                    for ct2 in range(2):
                        ct = half * 2 + ct2
                        nc.tensor.matmul(
                            cen_ps[:, ct2, :],
                            gt32[:, nt, ct * 128:(ct + 1) * 128],
                            gt32[:, nt, :],
                            start=(nt == 0),
                            stop=(nt == 7),
                        )
                nc.vector.tensor_copy(
                    cen_sb[:, half * 2:(half + 1) * 2], cen_ps[:]
                )

            # ---- f branch convs, Gram interleaved ----
            conv_cot(0)
            gram_half(0)
            conv_cot(1)
            gram_half(1)
            nc.sync.dma_start(CENP.ap().rearrange("t p c -> p t c"), cen_sb[:])
            conv_cot(2)
            conv_cot(3)

            # ---- q/k : packed f32r matmul (q rows 0:64, k rows 64:128) ----
            qk_ps = pspool.tile([128, 2, 512], F32, tag="qkcen", bufs=1)
            for ck in range(2):
                for cit in range(4):
                    nc.tensor.matmul(
                        qk_ps[:, ck, :],
                        qkwt[:, cit, :],
                        fgv32[:, cit, ck * 512:(ck + 1) * 512],
                        start=(cit == 0),
                        stop=(cit == 3),
                    )
            qk_sb = opool.tile([128, NPIX], F32, tag="qk_sb")
            nc.vector.tensor_scalar(
                out=qk_sb[:], in0=qk_ps.rearrange("p a b -> p (a b)"),
                scalar1=qkb[:], scalar2=None, op0=ALU.add,
            )
            nc.sync.dma_start(QK[:], qk_sb[:])

            # ---- vT[n, c] (no bias: folded in launch2) ----
            for nt in range(8):
                vps = pspool.tile([128, 512], F32, tag="small")
                for cit in range(4):
                    nc.tensor.matmul(
                        vps[:],
                        fgv[:, cit, nt * 128:(nt + 1) * 128],
                        vwt[:, cit, :],
                        start=(cit == 0),
                        stop=(cit == 3),
                    )
                vt_sb = opool.tile([128, 512], BF16, tag="vt_sb")
                nc.vector.tensor_copy(vt_sb[:], vps[:])
                nc.sync.dma_start(VT[nt], vt_sb[:])

    nc.compile()
    return nc


# --------------------------------------------------------------------------
# launch 2: PAM (transpose-free) + CAM + output convs + classifiers + fusion
# --------------------------------------------------------------------------

def build_launch2():
    nc = bacc.Bacc(None, target_bir_lowering=False)

    KAUG = nc.dram_tensor("KAUG", [65, N], F32, kind="ExternalInput")
    QAUG = nc.dram_tensor("QAUG", [65, NPIXH], F32, kind="ExternalInput")
    KB16 = nc.dram_tensor("KB16", [64, N], BF16, kind="ExternalInput")
    QB16 = nc.dram_tensor("QB16", [64, NPIXH], BF16, kind="ExternalInput")
    VT2 = nc.dram_tensor("VT2", [32, 128, 512], BF16, kind="ExternalInput")
    CEN = nc.dram_tensor("CEN", [4, 128, 512], F32, kind="ExternalInput")
    FH = nc.dram_tensor("FH", [4, 128, HR, W], BF16, kind="ExternalInput")
    GH = nc.dram_tensor("GH", [4, 128, HR, W], BF16, kind="ExternalInput")
    W2T = nc.dram_tensor("W2T", [2, 4, 128, 4, 9, 128], BF16, kind="ExternalInput")
    OSC = nc.dram_tensor("OSC", [128, 8], F32, kind="ExternalInput")
    OSH = nc.dram_tensor("OSH", [128, 8], F32, kind="ExternalInput")
    CLSW = nc.dram_tensor("CLSW", [3, 4, 128, NCLS], BF16, kind="ExternalInput")
    CLSB = nc.dram_tensor("CLSB", [NCLS, 3], F32, kind="ExternalInput")
    VB = nc.dram_tensor("VB", [128, 4], F32, kind="ExternalInput")
    GAM = nc.dram_tensor("GAM", [1, 2], F32, kind="ExternalInput")
    MSKB = nc.dram_tensor("MSKB", [128, NPIXH], BF16, kind="ExternalInput")

    OUT = nc.dram_tensor("OUT", [3, NCLS, RS, W], F32, kind="ExternalOutput")

    with tile.TileContext(nc) as tc:
        with (
            tc.tile_pool(name="singles", bufs=1) as singles,
            tc.tile_pool(name="w2p", bufs=2) as w2p,
            tc.tile_pool(name="work", bufs=2) as work,
            tc.tile_pool(name="cols", bufs=4) as cols,
            tc.tile_pool(name="pspool", bufs=2, space="PSUM") as pspool,
        ):
            # ---- input DMAs, roughly in order of first use ----
            qb16 = singles.tile([64, NPIXH], BF16)
            nc.sync.dma_start(qb16[:], QB16[:])
            kb16 = singles.tile([64, N], BF16)
            nc.sync.dma_start(kb16[:], KB16[:])
            cen = singles.tile([128, 4, 512], F32)
            nc.sync.dma_start(cen[:], CEN.ap().rearrange("t p c -> p t c"))
            gh = singles.tile([128, 4, HR, W], BF16)
            nc.sync.dma_start(gh[:], GH.ap().rearrange("t p r c -> p t r c"))
            gam_pa = singles.tile([128, 1], F32)
            nc.sync.dma_start(
                gam_pa[:],
                bass.AP(tensor=GAM.ap().tensor, offset=0, ap=[[0, 128], [1, 1]]),
            )
            gam_ca = singles.tile([128, 1], F32)
            nc.sync.dma_start(
                gam_ca[:],
                bass.AP(tensor=GAM.ap().tensor, offset=1, ap=[[0, 128], [1, 1]]),
            )
            vb = singles.tile([128, 4], F32)
            nc.sync.dma_start(vb[:], VB[:])
            osc = singles.tile([128, 8], F32)
            nc.sync.dma_start(osc[:], OSC[:])
            osh = singles.tile([128, 8], F32)
            nc.sync.dma_start(osh[:], OSH[:])
            kaug = singles.tile([65, N], F32)
            nc.sync.dma_start(kaug[:], KAUG[:])
            qaug = singles.tile([65, NPIXH], F32)
            nc.sync.dma_start(qaug[0:64, :], QAUG[0:64, :])
            mskb = singles.tile([128, NPIXH], BF16)
            nc.sync.dma_start(mskb[:], MSKB[:])
            fh = singles.tile([128, 4, HR, W], BF16)
            nc.sync.dma_start(fh[:], FH.ap().rearrange("t p r c -> p t r c"))
            vt = singles.tile([128, 32, 512], BF16)
            nc.sync.dma_start(vt[:], VT2.ap().rearrange("n p c -> p n c"))
            clsw = singles.tile([128, 3, 4, NCLS], BF16)
            nc.sync.dma_start(clsw[:], CLSW.ap().rearrange("w t p c -> p w t c"))
            ones1 = singles.tile([1, 128], F32)
            nc.sync.dma_start(ones1[:], KAUG[64:65, 0:128])
            clsb = singles.tile([NCLS, 3], F32)
            nc.sync.dma_start(clsb[:], CLSB[:])

            identf = singles.tile([128, 128], F32)
            make_identity(nc, identf[:])
            identb = singles.tile([128, 128], BF16)
            make_identity(nc, identb[:])
            onesj = singles.tile([128, 1], BF16)
            nc.vector.memset(onesj[:], 1.0)

            ghv = gh.rearrange("p t r c -> p t (r c)")
            fhv = fh.rearrange("p t r c -> p t (r c)")

            gvb = singles.tile([128, 4], F32)
            nc.vector.tensor_scalar(
                out=gvb[:], in0=vb[:], scalar1=gam_pa[:], scalar2=None,
                op0=ALU.mult,
            )

            negm9 = singles.tile([128, 9], F32)
            feat_bf = singles.tile([128, 2, 4, RS, W], BF16)
            pT = singles.tile([128, 32, 384], BF16)
            pabuf = singles.tile([128, 4, HR, W + 2], BF16)
            cabuf = singles.tile([128, 4, HR, W + 2], BF16)
            nc.vector.memset(cabuf[:], 0.0)

            # ---- PAM pass 1: bf16 row-major energies, per-query max ----
            def pass1_it(it):
                nmx8 = cols.tile([128, 8], F32, tag="nmx8")
                for jc in range(8):
                    eps = pspool.tile([128, 512], F32, tag="e")
                    nc.tensor.matmul(
                        eps[:],
                        qb16[:, it * 128:(it + 1) * 128],
                        kb16[:, jc * 512:(jc + 1) * 512],
                        start=True, stop=True,
                    )
                    nc.vector.tensor_reduce(
                        out=nmx8[:, jc:jc + 1], in_=eps[:], op=ALU.max,
                        axis=mybir.AxisListType.X, negate=True,
                    )
                nc.vector.tensor_reduce(
                    out=negm9[:, it:it + 1], in_=nmx8[:], op=ALU.min,
                    axis=mybir.AxisListType.X,
                )

            def third_negm(k):
                # -M into qaug row 64 (3 single-partition DMAs)
                tpn = pspool.tile([3, 128], F32, tag="e")
                nc.tensor.transpose(tpn[:], negm9[:, 3 * k:3 * k + 3], identf[:])
                rowr = work.tile([3, 128], F32, tag="rowr")
                nc.vector.tensor_copy(rowr[:], tpn[:])
                for a in range(3):
                    nc.sync.dma_start(
                        qaug[64:65, 384 * k + 128 * a:384 * k + 128 * (a + 1)],
                        rowr[a:a + 1, :],
                    )

            def ca_branch():
                E_sb = singles.tile([128, 4, 512], BF16)
                Scol = singles.tile([128, 4], F32)
                for ct in range(4):
                    mn = cols.tile([128, 1], F32, tag="camn")
                    nc.vector.tensor_reduce(
                        out=mn[:], in_=cen[:, ct, :], op=ALU.min,
                        axis=mybir.AxisListType.X,
                    )
                    nc.scalar.activation(
                        out=E_sb[:, ct, :], in_=cen[:, ct, :], func=AF.Exp,
                        bias=mn[:], scale=-1.0, accum_out=Scol[:, ct:ct + 1],
                    )
                grS = singles.tile([128, 4], F32)
                nc.vector.reciprocal(grS[:], Scol[:])
                nc.vector.tensor_scalar(
                    out=grS[:], in0=grS[:], scalar1=gam_ca[:], scalar2=None,
                    op0=ALU.mult,
                )
                ET = singles.tile([128, 4, 512], BF16)
                for ct in range(4):
                    for dt in range(4):
                        tp = pspool.tile([128, 128], BF16, tag="e")
                        nc.tensor.transpose(
                            tp[:], E_sb[:, ct, dt * 128:(dt + 1) * 128],
                            identb[:],
                        )
                        nc.vector.tensor_copy(
                            ET[:, dt, ct * 128:(ct + 1) * 128], tp[:]
                        )
                for ck in range(3):
                    px0 = ck * 384
                    ca_ps = pspool.tile([128, 4, 512], F32, tag="pa", bufs=1)
                    for ct in range(4):
                        for dt in range(4):
                            nc.tensor.matmul(
                                ca_ps[:, ct, :384],
                                ET[:, dt, ct * 128:(ct + 1) * 128],
                                ghv[:, dt, px0:px0 + 384],
                                start=(dt == 0),
                                stop=(dt == 3),
                            )
                    for ct in range(4):
                        tmp = work.tile([128, 384], F32, tag="catmp")
                        nc.vector.tensor_scalar(
                            out=tmp[:], in0=ca_ps[:, ct, :384],
                            scalar1=grS[:, ct:ct + 1], scalar2=None,
                            op0=ALU.mult,
                        )
                        nc.vector.tensor_add(
                            cabuf[:, ct, ck * 6:(ck + 1) * 6, 1:1 + W],
                            tmp.rearrange("p (r c) -> p r c", c=W),
                            ghv[:, ct, px0:px0 + 384]
                            .rearrange("p (r c) -> p r c", c=W),
                        )

            def w2_load(br, cot):
                w2v = w2p.tile([128, 4, 9, 128], BF16, tag="w2")
                nc.sync.dma_start(w2v[:], W2T[br, cot])
                return w2v

            def conv_group(br, buf, cot, rb, w2v):
                acc = pspool.tile([128, 8, W], F32, tag="cv")
                nmm = 0
                for cit in range(4):
                    wq = w2v[:, cit]
                    for dd in range(9):
                        dy, dx = dd // 3, dd % 3
                        r0 = rb * 8 + dy
                        nc.tensor.matmul(
                            acc[:],
                            wq[:, dd, :],
                            buf[:, cit, r0:r0 + 8, dx:dx + W],
                            start=(nmm == 0),
                            stop=(nmm == 35),
                        )
                        nmm += 1
                nc.scalar.activation(
                    out=feat_bf[:, br, cot, rb * 8:(rb + 1) * 8, :],
                    in_=acc[:],
                    func=AF.Relu,
                    bias=osh[:, br * 4 + cot:br * 4 + cot + 1],
                    scale=osc[:, br * 4 + cot:br * 4 + cot + 1],
                )

            featv = feat_bf.rearrange("p b t r c -> p b t (r c)")

            def classifier_ck(which, ck):
                # which 0: fusion (paf + caf through fW), 1: pa, 2: ca
                # ck 0 covers rows 0..7 (rb0 features), ck 1 rows 8..15
                sl = slice(ck * 512, (ck + 1) * 512)
                cls_ps = pspool.tile([NCLS, 512], F32, tag="cv")
                if which == 0:
                    nmm = 0
                    for br in range(2):
                        for cit in range(4):
                            nc.tensor.matmul(
                                cls_ps[:],
                                clsw[:, 0, cit, :],
                                featv[:, br, cit, sl],
                                start=(nmm == 0), stop=(nmm == 7),
                            )
                            nmm += 1
                else:
                    br = which - 1
                    for cit in range(4):
                        nc.tensor.matmul(
                            cls_ps[:],
                            clsw[:, which, cit, :],
                            featv[:, br, cit, sl],
                            start=(cit == 0), stop=(cit == 3),
                        )
                out_sb = work.tile([NCLS, 512], F32, tag="out_sb")
                nc.vector.tensor_scalar(
                    out=out_sb[:], in0=cls_ps[:],
                    scalar1=clsb[:, which:which + 1], scalar2=None,
                    op0=ALU.add,
                )
                nc.sync.dma_start(
                    OUT[which, :, ck * 8:(ck + 1) * 8, :]
                    .rearrange("p r c -> p (r c)"),
                    out_sb[:],
                )

            # ---- pass1 its interleaved with CA branch + cao conv ----
            cao_groups = [(cot, rb) for cot in range(4) for rb in range(2)]
            w2v_live = {}

            def cao_group(g):
                cot, rb = cao_groups[g]
                if rb == 0:
                    w2v_live[cot] = w2_load(1, cot)
                conv_group(1, cabuf, cot, rb, w2v_live[cot])

            pass1_it(0)
            pass1_it(1)
            ca_branch()
            pass1_it(2)
            third_negm(0)
            for it in range(3, 9):
                pass1_it(it)
                if it == 5:
                    third_negm(1)
                elif it == 8:
                    third_negm(2)
                cao_group(it - 3)

            nc.vector.memset(pabuf[:], 0.0)

            # FM = (f + gamma*vb) * mask  (residual term of the PA epilogue)
            FM = singles.tile([128, 4, NPIXH], BF16)
            for ct in range(4):
                nc.vector.tensor_scalar(
                    out=FM[:, ct, :], in0=fhv[:, ct, :],
                    scalar1=gvb[:, ct:ct + 1], scalar2=None, op0=ALU.add,
                )
                nc.vector.tensor_mul(FM[:, ct, :], FM[:, ct, :], mskb[:])

            def cls_finish(which, ck, cls_ps):
                out_sb = work.tile([NCLS, 512], F32, tag="out_sb")
                nc.vector.tensor_scalar(
                    out=out_sb[:], in0=cls_ps[:],
                    scalar1=clsb[:, which:which + 1], scalar2=None,
                    op0=ALU.add,
                )
                nc.sync.dma_start(
                    OUT[which, :, ck * 8:(ck + 1) * 8, :]
                    .rearrange("p r c -> p (r c)"),
                    out_sb[:],
                )

            def pao_block(rb, ck):
                # pao conv groups with the fusion/pa classifier matmuls
                # folded in per cot: after the last conv only bias+DMA is
                # left. Accumulators live in the then-idle "e" PSUM ring.
                sl = slice(ck * 512, (ck + 1) * 512)
                cls0 = pspool.tile([NCLS, 512], F32, tag="e", name="cls0")
                cls1 = pspool.tile([NCLS, 512], F32, tag="e", name="cls1")
                for cit in range(4):    # cao half of the fusion head
                    nc.tensor.matmul(
                        cls0[:], clsw[:, 0, cit, :], featv[:, 1, cit, sl],
                        start=(cit == 0), stop=False,
                    )
                for cot in range(4):
                    if cot >= 2:
                        w2v_live[cot] = w2_load(0, cot)
                    conv_group(0, pabuf, cot, rb, w2v_live[cot])
                    nc.tensor.matmul(
                        cls0[:], clsw[:, 0, cot, :], featv[:, 0, cot, sl],
                        start=False, stop=(cot == 3),
                    )
                    nc.tensor.matmul(
                        cls1[:], clsw[:, 1, cot, :], featv[:, 0, cot, sl],
                        start=(cot == 0), stop=(cot == 3),
                    )
                cls_finish(0, ck, cls0)
                cls_finish(1, ck, cls1)

            # ---- PAM pass 2 + PA accumulation, per third ----
            def eT_exp(k, jt):
                eT = pspool.tile([128, 384], F32, tag="e")
                nc.tensor.matmul(
                    eT[:],
                    kaug[:, jt * 128:(jt + 1) * 128],
                    qaug[:, 384 * k:384 * (k + 1)],
                    start=True, stop=True,
                )
                nc.scalar.activation(
                    out=pT[:, jt, :], in_=eT[:], func=AF.Exp,
                    bias=0.0, scale=1.0,
                )

            for k in range(NT3):
                q_sl = slice(384 * k, 384 * (k + 1))
                if k == 0:
                    for jt in range(16):
                        eT_exp(0, jt)
                    cao_group(6)
                    for jt in range(16, 32):
                        eT_exp(0, jt)
                    cao_group(7)
                    classifier_ck(2, 0)     # ca head: only needs cao output
                    classifier_ck(2, 1)
                    w2v_live[0] = w2_load(0, 0)
                    w2v_live[1] = w2_load(0, 1)
                pa_ps = pspool.tile([128, 4, 512], F32, tag="pa", bufs=1)
                dn = pspool.tile([1, 384], F32, tag="cv")
                for jt in range(32):
                    for ct in range(4):
                        nc.tensor.matmul(
                            pa_ps[:, ct, :384],
                            vt[:, jt, ct * 128:(ct + 1) * 128],
                            pT[:, jt, :],
                            start=(jt == 0),
                            stop=(jt == 31),
                        )
                    nc.tensor.matmul(
                        dn[:], onesj[:], pT[:, jt, :],
                        start=(jt == 0), stop=(jt == 31),
                    )
                    if k + 1 < NT3:
                        eT_exp(k + 1, jt)
                # R = gamma * mask / S, broadcast over channel partitions
                R3r = work.tile([1, 384], F32, tag="R3r")
                nc.vector.reciprocal(R3r[:], dn[:])
                nc.vector.tensor_scalar(
                    out=R3r[:], in0=R3r[:], scalar1=gam_pa[0:1, :],
                    scalar2=None, op0=ALU.mult,
                )
                nc.vector.tensor_mul(R3r[:], R3r[:], mskb[0:1, q_sl])
                rb_ps = pspool.tile([128, 384], F32, tag="e")
                nc.tensor.matmul(rb_ps[:], ones1[:], R3r[:],
                                 start=True, stop=True)
                Rbm = work.tile([128, 384], F32, tag="Rbm")
                nc.vector.tensor_copy(Rbm[:], rb_ps[:])
                # epilogue: pabuf = pa * R + FM
                for ct in range(4):
                    tmp2 = work.tile([128, 384], F32, tag="patmp")
                    nc.vector.tensor_mul(tmp2[:], pa_ps[:, ct, :384], Rbm[:])
                    nc.vector.tensor_add(
                        pabuf[:, ct, k * 6:(k + 1) * 6, 1:1 + W],
                        tmp2.rearrange("p (r c) -> p r c", c=W),
                        FM[:, ct, q_sl].rearrange("p (r c) -> p r c", c=W),
                    )
                if k == 1:
                    # pao rb0: needs pabuf rows 0..9 (thirds 0 and 1)
                    pao_block(0, 0)
                elif k == 2:
                    w2v_live[0] = w2_load(0, 0)
                    w2v_live[1] = w2_load(0, 1)

            # pao rb1 (pabuf rows 8..17)
            pao_block(1, 1)

    nc.compile()
    return nc


# --------------------------------------------------------------------------
# host-side preparation and glue
# --------------------------------------------------------------------------

_CACHE = {}


def _get_kernels():
    if "nc1" not in _CACHE:
        _CACHE["nc1"] = build_launch1()
        _CACHE["nc2"] = build_launch2()
    return _CACHE["nc1"], _CACHE["nc2"]


def _fold_bn(g, b, m, v, conv_b):
    scale = g / np.sqrt(v + EPS)
    shift = (conv_b - m) * scale + b
    return scale.astype(np.float32), shift.astype(np.float32)


def _prep_launch1(x, paW, pab, pa_bn, caW, cab, ca_bn, qW, qb, kW, kb, vW):
    """Build the 8 per-core input maps for launch 1."""
    W1 = np.concatenate([paW, caW], axis=0)            # (1024, 2048, 3, 3)
    w1t = np.ascontiguousarray(
        np.transpose(W1.reshape(8, 128, 16, 128, 3, 3), (0, 3, 2, 4, 5, 1))
    ).reshape(8, 128, 16, 9, 128).astype(np.float32)

    sc_f, sh_f = _fold_bn(*pa_bn, pab)
    sc_g, sh_g = _fold_bn(*ca_bn, cab)
    fgsc = np.concatenate([sc_f, sc_g]).reshape(8, 128).T.copy()   # (128, 8)
    fgsh = np.concatenate([sh_f, sh_g]).reshape(8, 128).T.copy()

    qkW = np.concatenate([qW[:, :, 0, 0], kW[:, :, 0, 0]], axis=0)  # (128, 512)
    qkwt = np.ascontiguousarray(
        qkW.T.reshape(4, 128, 128)
    ).astype(np.float32)                               # [cit, ci, co]
    qkb_ = np.concatenate([qb, kb]).reshape(128, 1).astype(np.float32)
    vwt = np.ascontiguousarray(
        vW[:, :, 0, 0].T.reshape(4, 128, 512)
    ).astype(bf16)
    idr = np.eye(128, dtype=np.float32)

    # padded input slices, pre-transposed to partition-major layout
    xpad = np.zeros((B, CIN, H + 2, W + 2), dtype=np.float32)
    xpad[:, :, 1:H + 1, 1:W + 1] = x.astype(np.float32)

    in_maps = []
    for c in range(NCORE):
        b_, s_ = divmod(c, S)
        rows = slice(s_ * RS, s_ * RS + HR)            # in padded coords
        xp = np.ascontiguousarray(
            xpad[b_, :, rows, :].reshape(16, 128, HR, W + 2)
            .transpose(1, 0, 2, 3)
        )
        in_maps.append({
            "XP": xp, "W1T": w1t, "FGSC": fgsc, "FGSH": fgsh,
            "QKWT": qkwt, "QKB": qkb_, "VWT": vwt, "IDR": idr,
        })
    return in_maps


def _prep_launch2(r1, paoW, paob, pao_bn, caoW, caob, cao_bn,
                  paclsW, paclsb, caclsW, caclsb, fW, fb,
                  vb, pam_gamma, cam_gamma):
    """Reshuffle launch-1 outputs and build launch-2 input maps."""
    f_full = np.zeros((B, 4, 128, H, W), dtype=np.float32)
    g_full = np.zeros((B, 4, 128, H, W), dtype=np.float32)
    q_full = np.zeros((B, 64, H, W), dtype=np.float32)
    k_full = np.zeros((B, 64, H, W), dtype=np.float32)
    vt_full = np.zeros((B, 32, 128, 512), dtype=bf16)
    cen_full = np.zeros((B, 4, 128, 512), dtype=np.float32)
    for c in range(NCORE):
        b_, s_ = divmod(c, S)
        r = r1[c]
        rows = slice(s_ * RS, (s_ + 1) * RS)
        f_full[b_, :, :, rows, :] = r["FG"][0:4]
        g_full[b_, :, :, rows, :] = r["FG"][4:8]
        qk = r["QK"].reshape(128, RS, W)
        q_full[b_, :, rows, :] = qk[0:64]
        k_full[b_, :, rows, :] = qk[64:128]
        vt_full[b_, s_ * 8:(s_ + 1) * 8] = r["VT"]
        cen_full[b_] += r["CENP"]

    w2 = np.stack([paoW, caoW])                        # (2, 512, 512, 3, 3)
    w2t = np.ascontiguousarray(
        np.transpose(w2.reshape(2, 4, 128, 4, 128, 3, 3), (0, 1, 4, 3, 5, 6, 2))
    ).reshape(2, 4, 128, 4, 9, 128).astype(bf16)

    sc_p, sh_p = _fold_bn(*pao_bn, paob)
    sc_c, sh_c = _fold_bn(*cao_bn, caob)
    osc = np.concatenate([sc_p, sc_c]).reshape(8, 128).T.copy()
    osh = np.concatenate([sh_p, sh_c]).reshape(8, 128).T.copy()

    clsw = np.stack([
        fW[:, :, 0, 0], paclsW[:, :, 0, 0], caclsW[:, :, 0, 0]
    ])                                                 # (3, 19, 512)
    clsw_t = np.ascontiguousarray(
        np.transpose(clsw.reshape(3, NCLS, 4, 128), (0, 2, 3, 1))
    ).astype(bf16)                                     # (3, 4, 128, 19)
    clsb = np.stack([fb, paclsb, caclsb], axis=1).astype(np.float32)  # (19, 3)

    vb_t = vb.reshape(4, 128).T.copy().astype(np.float32)             # (128, 4)
    gam = np.array([[float(pam_gamma[0]), float(cam_gamma[0])]], np.float32)

    in_maps = []
    for c in range(NCORE):
        b_, s_ = divmod(c, S)
        r0 = s_ * RS - 1                               # first halo row
        # halo slices with zero pad
        fhs = np.zeros((4, 128, HR, W), dtype=bf16)
        ghs = np.zeros((4, 128, HR, W), dtype=bf16)
        qaug = np.zeros((65, NPIXH), dtype=np.float32)
        msk = np.zeros((HR, W), dtype=np.float32)
        lo, hi = max(r0, 0), min(r0 + HR, H)
        fhs[:, :, lo - r0:hi - r0, :] = f_full[b_, :, :, lo:hi, :].astype(bf16)
        ghs[:, :, lo - r0:hi - r0, :] = g_full[b_, :, :, lo:hi, :].astype(bf16)
        qaug[0:64].reshape(64, HR, W)[:, lo - r0:hi - r0, :] = \
            q_full[b_, :, lo:hi, :]
        msk[lo - r0:hi - r0, :] = 1.0
        mskb = np.broadcast_to(
            msk.reshape(1, NPIXH).astype(bf16), (128, NPIXH)
        ).copy()
        kaug = np.concatenate(
            [k_full[b_].reshape(64, N), np.ones((1, N), np.float32)], axis=0
        )
        in_maps.append({
            "KAUG": kaug, "QAUG": qaug,
            "KB16": kaug[0:64].astype(bf16), "QB16": qaug[0:64].astype(bf16),
            "VT2": vt_full[b_], "CEN": cen_full[b_],
            "FH": fhs, "GH": ghs,
            "W2T": w2t, "OSC": osc, "OSH": osh,
            "CLSW": clsw_t, "CLSB": clsb, "VB": vb_t, "GAM": gam,
            "MSKB": mskb,
        })
    return in_maps


def kernel(x, paW, pab, pa_g, pa_b, pa_m, pa_v,
           qW, qb, kW, kb, vW, vb, pam_gamma,
           paoW, paob, pao_g, pao_b, pao_m, pao_v, paclsW, paclsb,
           caW, cab, ca_g, ca_b, ca_m, ca_v, cam_gamma,
           caoW, caob, cao_g, cao_b, cao_m, cao_v, caclsW, caclsb,
           fW, fb, _profile=False):
    nc1, nc2 = _get_kernels()

    im1 = _prep_launch1(
        np.asarray(x), np.asarray(paW), np.asarray(pab),
        (np.asarray(pa_g), np.asarray(pa_b), np.asarray(pa_m), np.asarray(pa_v)),
        np.asarray(caW), np.asarray(cab),
        (np.asarray(ca_g), np.asarray(ca_b), np.asarray(ca_m), np.asarray(ca_v)),
        np.asarray(qW), np.asarray(qb), np.asarray(kW), np.asarray(kb),
        np.asarray(vW),
    )
    res1 = run_bass_kernel_spmd(nc1, im1, core_ids=list(range(NCORE)),
                                trace=_profile)
    t1 = res1.exec_time_ns

    im2 = _prep_launch2(
        res1.results,
        np.asarray(paoW), np.asarray(paob),
        (np.asarray(pao_g), np.asarray(pao_b), np.asarray(pao_m), np.asarray(pao_v)),
        np.asarray(caoW), np.asarray(caob),
        (np.asarray(cao_g), np.asarray(cao_b), np.asarray(cao_m), np.asarray(cao_v)),
        np.asarray(paclsW), np.asarray(paclsb),
        np.asarray(caclsW), np.asarray(caclsb),
        np.asarray(fW), np.asarray(fb),
        np.asarray(vb), np.asarray(pam_gamma), np.asarray(cam_gamma),
    )
    res2 = run_bass_kernel_spmd(nc2, im2, core_ids=list(range(NCORE)),
                                trace=_profile)
    t2 = res2.exec_time_ns

    fusion = np.zeros((B, NCLS, H, W), dtype=np.float32)
    pa_out = np.zeros((B, NCLS, H, W), dtype=np.float32)
    ca_out = np.zeros((B, NCLS, H, W), dtype=np.float32)
    for c in range(NCORE):
        b_, s_ = divmod(c, S)
        rows = slice(s_ * RS, (s_ + 1) * RS)
        o = res2.results[c]["OUT"]
        fusion[b_, :, rows, :] = o[0]
        pa_out[b_, :, rows, :] = o[1]
        ca_out[b_, :, rows, :] = o[2]

    if _profile:
        kernel.last_exec_ns = (t1, t2)
        kernel.last_results = (res1, res2)
    return (fusion, pa_out, ca_out)
